# revision 7
# baseline (speedup 1.0000x reference)
"""Causal multi-head attention (B=2, S=2048, D=1024, H=16) on 8 trn2 cores.

Sharding: core c handles batch b = c//4 and heads 4*(c%4) .. 4*(c%4)+3.
Each core:
  - projects its x_b (host-pretransposed to [D, S]) through its Wqkv column
    slice to Q^T/K^T (per head-pair tiles) and V (natural layout, with a
    fused ones-column for softmax denominators),
  - computes causal attention per head entirely in "transposed" layout
    (scores^T = K @ Q^T chunks via PE row-tiled head pairs, exp on ScalarE,
    diagonal-chunk masks via gpsimd affine_select, A^T V via PE with the
    ones-row producing the denominator),
  - projects the concatenated per-head outputs through its Wout row slice
    into a partial [S, D] for batch b,
  - ReduceScatters partials (add) across its 4-core batch group; each core
    emits rows 512*(c%4) .. +512 of the final output for batch b.
Host assembles the 8 [512, 1024] shards into (2, 2048, 1024).

Matmuls run in float32r (single-pass PE mode, ~1e-3 rel err, 4x faster
than true fp32); set _USE_F32R = False to fall back to full fp32.
"""

import sys

for _p in ("/opt/trn_rl_repo", "/opt/pypackages"):
    if _p not in sys.path:
        sys.path.insert(0, _p)

import numpy as np

import concourse.bass as bass
import concourse.mybir as mybir
import concourse.tile as tile
from concourse import bacc
from concourse.bass_utils import run_bass_kernel_spmd

B = 2
S = 2048
D = 1024
H = 16
DH = 64
HPC = 4            # heads per core
NCORES = 8
SB = 512           # q block (matmul moving dim)
KC = 128           # k chunk (contraction tile)
NSB = S // SB      # 4 q-blocks
NKC = S // KC      # 16 k-chunks
NDC = D // KC      # 8 contraction chunks for the projections

_USE_F32R = True

_compiled = None


def _dt_mm(nc):
    return mybir.dt.float32r if _USE_F32R else mybir.dt.float32


def _mm_dma(nc):
    # f32 -> f32r loads must go through gpsimd (casting DMA)
    return nc.gpsimd if _USE_F32R else nc.sync


def _build():
    f32 = mybir.dt.float32
    nc = bacc.Bacc(None, target_bir_lowering=False)
    fr = _dt_mm(nc)

    xt = nc.declare_dram_parameter("xt", [D, S], f32, isOutput=False)
    wqk = nc.declare_dram_parameter("wqk", [D, 4 * KC], f32, isOutput=False)
    wv = nc.declare_dram_parameter("wv", [D, HPC * DH], f32, isOutput=False)
    wout = nc.declare_dram_parameter("wout", [HPC * DH, D], f32, isOutput=False)
    bqk = nc.declare_dram_parameter("bqk", [4 * KC], f32, isOutput=False)
    bv = nc.declare_dram_parameter("bv", [HPC * DH], f32, isOutput=False)
    bo4 = nc.declare_dram_parameter("bo4", [D], f32, isOutput=False)
    vones = nc.declare_dram_parameter("vones", [KC, NKC], f32, isOutput=False)
    out_ext = nc.declare_dram_parameter("out", [SB, D], f32, isOutput=True)

    partial = nc.dram_tensor("partial", [S, D], f32)
    rs_out = nc.dram_tensor("rs_out", [SB, D], f32)

    with tile.TileContext(nc) as tc:
        with (
            tc.tile_pool(name="weights", bufs=1) as wp,
            tc.tile_pool(name="qkv", bufs=1) as qkvp,
            tc.tile_pool(name="obuf", bufs=1) as op,
            tc.tile_pool(name="misc", bufs=1) as mp,
            tc.tile_pool(name="evict", bufs=4) as ep,
        ):
            # ---- constants / weights -------------------------------------
            wqk_t = [wp.tile([KC, 4 * KC], fr, tag=f"wqk{k}", name=f"wqk{k}") for k in range(NDC)]
            wv_t = [wp.tile([KC, HPC * DH], fr, tag=f"wv{k}", name=f"wv{k}") for k in range(NDC)]
            wout_t = [wp.tile([KC, D], fr, tag=f"wout{k}", name=f"wout{k}") for k in range(2)]
            for k in range(NDC):
                _mm_dma(nc).dma_start(out=wqk_t[k][:], in_=wqk[k * KC:(k + 1) * KC, :])
                _mm_dma(nc).dma_start(out=wv_t[k][:], in_=wv[k * KC:(k + 1) * KC, :])
            for k in range(2):
                _mm_dma(nc).dma_start(out=wout_t[k][:], in_=wout[k * KC:(k + 1) * KC, :])

            bqk_t = [mp.tile([KC, 1], f32, tag=f"bqk{m}", name=f"bqk{m}") for m in range(4)]
            for m in range(4):
                nc.sync.dma_start(
                    out=bqk_t[m][:],
                    in_=bqk[m * KC:(m + 1) * KC].rearrange("(p o) -> p o", o=1),
                )
            bv_row = mp.tile([1, HPC * DH], f32, tag="bv_row")
            nc.sync.dma_start(out=bv_row[:], in_=bv.rearrange("(o f) -> o f", o=1))
            bv_bc = mp.tile([KC, HPC * DH], f32, tag="bv_bc")
            nc.gpsimd.partition_broadcast(out_ap=bv_bc[:], in_ap=bv_row[:])
            bo_row = mp.tile([1, D], f32, tag="bo_row")
            nc.sync.dma_start(out=bo_row[:], in_=bo4.rearrange("(o f) -> o f", o=1))
            bo_bc = mp.tile([KC, D], f32, tag="bo_bc")
            nc.gpsimd.partition_broadcast(out_ap=bo_bc[:], in_ap=bo_row[:])

            # ---- persistent activations ----------------------------------
            # QQ[p]: rows 0:64 = Q^T of head 2p, rows 64:128 = Q^T of head 2p+1
            QQ = [qkvp.tile([KC, S], fr, tag=f"QQ{p}", name=f"QQ{p}") for p in range(2)]
            KK = [qkvp.tile([KC, S], fr, tag=f"KK{p}", name=f"KK{p}") for p in range(2)]
            # V[h]: [128, 16*65]; chunk kc occupies cols kc*65..kc*65+64 with
            # V rows, col kc*65+64 stays 1.0 (denominator ones-row).
            V = [qkvp.tile([KC, NKC * (DH + 1)], fr, tag=f"V{h}", name=f"V{h}") for h in range(HPC)]
            for h in range(HPC):
                vv = V[h][:].rearrange("p (k c) -> p k c", c=DH + 1)
                _mm_dma(nc).dma_start(out=vv[:, :, DH], in_=vones[:, :])
            # O[p]: rows 0:64 = head 2p out^T (normalized), rows 64:128 = head 2p+1
            O = [op.tile([KC, S], fr, tag=f"O{p}", name=f"O{p}") for p in range(2)]

            # ---- phase 1: projections ------------------------------------
            with (
                tc.tile_pool(name="xbuf", bufs=12) as xp,
                tc.tile_pool(name="psum_proj", bufs=1, space="PSUM") as pp,
            ):
                for sblk in range(NSB):
                    xs = []
                    for k in range(NDC):
                        xtl = xp.tile([KC, SB], fr, tag="xt")
                        _mm_dma(nc).dma_start(
                            out=xtl[:],
                            in_=xt[k * KC:(k + 1) * KC, sblk * SB:(sblk + 1) * SB],
                        )
                        xs.append(xtl)
                    # Q^T / K^T : m-chunk 0 -> QQ[0], 1 -> KK[0], 2 -> QQ[1], 3 -> KK[1]
                    for m in range(4):
                        ps = pp.tile([KC, SB], f32, tag="ps_qk", bufs=4)
                        for k in range(NDC):
                            nc.tensor.matmul(
                                ps[:],
                                wqk_t[k][:, m * KC:(m + 1) * KC],
                                xs[k][:],
                                start=(k == 0),
                                stop=(k == NDC - 1),
                            )
                        dest = (QQ if m % 2 == 0 else KK)[m // 2]
                        nc.vector.tensor_scalar_add(
                            dest[:, sblk * SB:(sblk + 1) * SB],
                            ps[:],
                            bqk_t[m][:],
                        )
                    # V natural: lhsT = x^T chunk (stationary), rhs = Wv
                    for sc in range(SB // KC):
                        ps = pp.tile([KC, HPC * DH], f32, tag="ps_v", bufs=4)
                        for k in range(NDC):
                            nc.tensor.matmul(
                                ps[:],
                                xs[k][:, sc * KC:(sc + 1) * KC],
                                wv_t[k][:],
                                start=(k == 0),
                                stop=(k == NDC - 1),
                            )
                        kcg = sblk * (SB // KC) + sc  # global s-chunk index
                        for h in range(HPC):
                            nc.vector.tensor_add(
                                V[h][:, kcg * (DH + 1):kcg * (DH + 1) + DH],
                                ps[:, h * DH:(h + 1) * DH],
                                bv_bc[:, h * DH:(h + 1) * DH],
                            )

            # ---- phase 2: attention --------------------------------------
            with (
                tc.tile_pool(name="pbuf", bufs=1) as pb,
                tc.tile_pool(name="psum_att", bufs=1, space="PSUM") as pa,
            ):
                for p in range(2):
                    for qblk in range(NSB):
                        nkc = 4 * (qblk + 1)  # causal: k-chunks 0..nkc-1
                        P = [
                            pb.tile([KC, nkc * SB], fr, tag=f"P{hh}",
                                    name=f"P{hh}_{p}_{qblk}")
                            for hh in range(2)
                        ]
                        for kc in range(nkc):
                            for hh in range(2):  # head 2p + hh, row-tiled pair
                                r0 = hh * DH
                                ps = pa.tile([KC, SB], f32, tag=f"ps_s{hh}", bufs=3)
                                nc.tensor.matmul(
                                    ps[:],
                                    KK[p][r0:r0 + DH, kc * KC:(kc + 1) * KC],
                                    QQ[p][r0:r0 + DH, qblk * SB:(qblk + 1) * SB],
                                    start=True,
                                    stop=True,
                                )
                                nc.scalar.activation(
                                    P[hh][:, kc * SB:(kc + 1) * SB],
                                    ps[:],
                                    mybir.ActivationFunctionType.Exp,
                                    scale=1.0 / np.sqrt(DH),
                                )
                                d = kc - 4 * qblk
                                if d >= 0:  # diagonal chunk: zero where k > q
                                    nc.gpsimd.affine_select(
                                        out=P[hh][:, kc * SB:(kc + 1) * SB],
                                        in_=P[hh][:, kc * SB:(kc + 1) * SB],
                                        pattern=[[1, SB]],
                                        compare_op=mybir.AluOpType.is_ge,
                                        fill=0.0,
                                        base=-KC * d,
                                        channel_multiplier=-1,
                                    )
                        for hh in range(2):
                            h = 2 * p + hh
                            po = pa.tile([DH + 1, SB], f32, tag="ps_av", bufs=2)
                            for kc in range(nkc):
                                nc.tensor.matmul(
                                    po[:],
                                    V[h][:, kc * (DH + 1):(kc + 1) * (DH + 1)],
                                    P[hh][:, kc * SB:(kc + 1) * SB],
                                    start=(kc == 0),
                                    stop=(kc == nkc - 1),
                                )
                            rden = ep.tile([1, SB], f32, tag="rden")
                            nc.vector.reciprocal(rden[:], po[DH:DH + 1, :])
                            rden_bc = ep.tile([DH, SB], f32, tag="rden_bc")
                            nc.gpsimd.partition_broadcast(
                                out_ap=rden_bc[:], in_ap=rden[:]
                            )
                            r0 = hh * DH
                            nc.vector.tensor_mul(
                                O[p][r0:r0 + DH, qblk * SB:(qblk + 1) * SB],
                                po[0:DH, :],
                                rden_bc[:],
                            )

            # ---- phase 3: output projection ------------------------------
            with tc.tile_pool(name="psum_out", bufs=4, space="PSUM") as pu:
                for sc in range(NKC):
                    for nb in range(D // SB):
                        ps = pu.tile([KC, SB], f32, tag="ps_o")
                        for p in range(2):
                            nc.tensor.matmul(
                                ps[:],
                                O[p][:, sc * KC:(sc + 1) * KC],
                                wout_t[p][:, nb * SB:(nb + 1) * SB],
                                start=(p == 0),
                                stop=(p == 1),
                            )
                        ot = ep.tile([KC, SB], f32, tag="osb")
                        nc.vector.tensor_add(
                            ot[:], ps[:], bo_bc[:, nb * SB:(nb + 1) * SB]
                        )
                        nc.sync.dma_start(
                            out=partial[sc * KC:(sc + 1) * KC, nb * SB:(nb + 1) * SB],
                            in_=ot[:],
                        )

            # ---- phase 4: cross-core reduce ------------------------------
            nc.gpsimd.collective_compute(
                "ReduceScatter",
                mybir.AluOpType.add,
                replica_groups=[[0, 1, 2, 3], [4, 5, 6, 7]],
                ins=[partial[:]],
                outs=[rs_out[:]],
            )
            nc.sync.dma_start(out=out_ext[:], in_=rs_out[:])

    nc.compile()
    return nc


def _get_program():
    global _compiled
    if _compiled is None:
        _compiled = _build()
    return _compiled


def _shard_inputs(x, Wqkv, bqkv, Wout, bout):
    """Build the 8 per-core input maps (all host-side numpy)."""
    x = np.ascontiguousarray(x, dtype=np.float32)
    Wqkv = np.asarray(Wqkv, dtype=np.float32)
    bqkv = np.asarray(bqkv, dtype=np.float32)
    Wout = np.asarray(Wout, dtype=np.float32)
    bout = np.asarray(bout, dtype=np.float32)

    Wq = Wqkv[:, 0 * D:1 * D]
    Wk = Wqkv[:, 1 * D:2 * D]
    Wv_full = Wqkv[:, 2 * D:3 * D]
    bq = bqkv[0 * D:1 * D]
    bk = bqkv[1 * D:2 * D]
    bv_full = bqkv[2 * D:3 * D]

    in_maps = []
    for c in range(NCORES):
        b = c // 4
        h0 = HPC * (c % 4)
        heads = list(range(h0, h0 + HPC))
        xt = np.ascontiguousarray(x[b].T)  # [D, S]
        # m-chunks: [q_h0|q_h1], [k_h0|k_h1], [q_h2|q_h3], [k_h2|k_h3]
        cols = []
        bcols = []
        for pp in range(2):
            ha, hb = heads[2 * pp], heads[2 * pp + 1]
            cols += [Wq[:, ha * DH:(ha + 1) * DH], Wq[:, hb * DH:(hb + 1) * DH],
                     Wk[:, ha * DH:(ha + 1) * DH], Wk[:, hb * DH:(hb + 1) * DH]]
            bcols += [bq[ha * DH:(ha + 1) * DH], bq[hb * DH:(hb + 1) * DH],
                      bk[ha * DH:(ha + 1) * DH], bk[hb * DH:(hb + 1) * DH]]
        wqk_c = np.ascontiguousarray(np.concatenate(cols, axis=1))
        bqk_c = np.ascontiguousarray(np.concatenate(bcols))
        wv_c = np.ascontiguousarray(
            np.concatenate([Wv_full[:, h * DH:(h + 1) * DH] for h in heads], axis=1)
        )
        bv_c = np.ascontiguousarray(
            np.concatenate([bv_full[h * DH:(h + 1) * DH] for h in heads])
        )
        wout_c = np.ascontiguousarray(
            np.concatenate([Wout[h * DH:(h + 1) * DH, :] for h in heads], axis=0)
        )
        bo4_c = (bout * 0.25).astype(np.float32)  # each group member adds 1/4
        in_maps.append({
            "xt": xt, "wqk": wqk_c, "wv": wv_c, "wout": wout_c,
            "bqk": bqk_c, "bv": bv_c, "bo4": bo4_c,
            "vones": np.ones((KC, NKC), dtype=np.float32),
        })
    return in_maps


def run(inputs, trace=False, trace_kwargs=None):
    nc = _get_program()
    in_maps = _shard_inputs(**inputs)
    res = run_bass_kernel_spmd(
        nc, in_maps, list(range(NCORES)), trace=trace,
        **(trace_kwargs or {}),
    )
    out = np.empty((B, S, D), dtype=np.float32)
    for c in range(NCORES):
        b = c // 4
        r0 = SB * (c % 4)
        out[b, r0:r0 + SB, :] = res.results[c]["out"]
    return out, res


def kernel(**inputs):
    out, _ = run(inputs)
    return out


# revision 9
# speedup vs baseline: 1.0647x; 1.0647x over previous
"""Causal multi-head attention (B=2, S=2048, D=1024, H=16) on 8 trn2 cores.

Sharding: core c handles heads {2c, 2c+1} of BOTH batches (4 (b,h) pairs).
Per core:
  - project host-pretransposed x_b^T [D, S] (both batches) through the
    core's Wqkv column slice into Q^T/K^T head-pair tiles and V (natural
    layout, with a fused ones-column that makes the AV matmul emit softmax
    denominators),
  - causal attention per (batch, head) in transposed layout: scores^T =
    K Q^T chunks (PE row-tiled head pairs), exp on ScalarE, causal diagonal
    masks via gpsimd affine_select, A^T V on PE,
  - one 8-wide AllToAll redistributes head outputs so core c holds ALL 16
    heads of batch c//4 for sequence quarter c%4,
  - local projection through the full Wout emits final rows
    512*(c%4) .. +512 of batch c//4.
Host assembles the 8 [512, 1024] shards into (2, 2048, 1024).

Matmuls run in float32r (TF32-like single-pass PE mode, ~1e-3 rel err,
4x faster than true fp32). The PE rounds f32r inputs internally, so DRAM
inputs are declared float32r and DMA'd with the fast HW-DGE path with no
pre-rounding. Set _USE_F32R = False for full fp32.
"""

import sys

for _p in ("/opt/trn_rl_repo", "/opt/pypackages"):
    if _p not in sys.path:
        sys.path.insert(0, _p)

import numpy as np

import concourse.bass as bass
import concourse.mybir as mybir
import concourse.tile as tile
from concourse import bacc
from concourse.bass_utils import run_bass_kernel_spmd

B = 2
S = 2048
D = 1024
H = 16
DH = 64
NCORES = 8
SB = 512           # q block (matmul moving dim)
KC = 128           # k chunk (contraction tile)
NSB = S // SB      # 4 q-blocks
NKC = S // KC      # 16 k-chunks
NDC = D // KC      # 8 contraction chunks for the projections

_USE_F32R = True

_compiled = None


def _build():
    f32 = mybir.dt.float32
    fr = mybir.dt.float32r if _USE_F32R else f32
    nc = bacc.Bacc(None, target_bir_lowering=False)

    # host-blocked inputs: every [128, N] tile is contiguous in DRAM.
    # Matmul inputs are declared float32r: same 4-byte data, PE rounds
    # internally, and plain (non-casting) sync DMA is allowed.
    xt = nc.declare_dram_parameter("xt", [B, NSB, NDC, KC, SB], fr, isOutput=False)
    wqk = nc.declare_dram_parameter("wqk", [NDC, KC, 2 * KC], fr, isOutput=False)
    wv = nc.declare_dram_parameter("wv", [NDC, KC, 2 * KC], fr, isOutput=False)
    wout = nc.declare_dram_parameter("wout", [NDC, KC, D], fr, isOutput=False)
    bqk = nc.declare_dram_parameter("bqk", [2 * KC], f32, isOutput=False)
    bv = nc.declare_dram_parameter("bv", [2 * DH], f32, isOutput=False)
    bo = nc.declare_dram_parameter("bo", [D], f32, isOutput=False)
    vones = nc.declare_dram_parameter("vones", [KC, NKC], fr, isOutput=False)
    out_ext = nc.declare_dram_parameter("out", [SB, D], f32, isOutput=True)

    # AllToAll staging: block t -> core t gets my heads of batch t//4 for
    # s-quarter t%4.
    a2a_in = nc.dram_tensor("a2a_in", [NCORES, KC, SB], fr)
    a2a_out = nc.dram_tensor("a2a_out", [NCORES, KC, SB], fr)

    with tile.TileContext(nc) as tc:
        with (
            tc.tile_pool(name="qkv", bufs=1) as qkvp,
            tc.tile_pool(name="obuf", bufs=1) as op,
            tc.tile_pool(name="misc", bufs=1) as mp,
            tc.tile_pool(name="evict", bufs=1) as ep,
        ):
            # ---- small constants -----------------------------------------
            bqk_t = [mp.tile([KC, 1], f32, tag=f"bqk{m}", name=f"bqk{m}")
                     for m in range(2)]
            for m in range(2):
                nc.sync.dma_start(
                    out=bqk_t[m][:],
                    in_=bqk[m * KC:(m + 1) * KC].rearrange("(p o) -> p o", o=1),
                )
            bv_row = mp.tile([1, 2 * DH], f32, tag="bv_row")
            nc.sync.dma_start(out=bv_row[:], in_=bv.rearrange("(o f) -> o f", o=1))
            bv_bc = mp.tile([KC, 2 * DH], f32, tag="bv_bc")
            nc.gpsimd.partition_broadcast(out_ap=bv_bc[:], in_ap=bv_row[:])
            bo_row = mp.tile([1, D], f32, tag="bo_row")
            nc.sync.dma_start(out=bo_row[:], in_=bo.rearrange("(o f) -> o f", o=1))
            bo_bc = mp.tile([KC, D], f32, tag="bo_bc")
            nc.gpsimd.partition_broadcast(out_ap=bo_bc[:], in_ap=bo_row[:])

            # ---- persistent activations ----------------------------------
            # pair p = batch p with heads (2c, 2c+1).
            # QQ[p]: rows 0:64 = Q^T of head 2c, rows 64:128 = head 2c+1
            QQ = [qkvp.tile([KC, S], fr, tag=f"QQ{p}", name=f"QQ{p}") for p in range(2)]
            KK = [qkvp.tile([KC, S], fr, tag=f"KK{p}", name=f"KK{p}") for p in range(2)]
            # V[2p+hh]: [128, 16*65]; chunk kc at cols kc*65..+64; col 64: 1.0
            V = [qkvp.tile([KC, NKC * (DH + 1)], fr, tag=f"V{v}", name=f"V{v}")
                 for v in range(4)]
            for v in range(4):
                vv = V[v][:].rearrange("p (k c) -> p k c", c=DH + 1)
                nc.sync.dma_start(out=vv[:, :, DH], in_=vones[:, :])
            # O[p]: rows 0:64 = head 2c out^T (normalized), 64:128 = head 2c+1
            O = [op.tile([KC, S], fr, tag=f"O{p}", name=f"O{p}") for p in range(2)]

            # ---- phase 1: projections ------------------------------------
            with (
                tc.tile_pool(name="pjw", bufs=1) as wp,
                tc.tile_pool(name="xbuf", bufs=24) as xp,
                tc.tile_pool(name="psum_proj", bufs=1, space="PSUM") as pp,
            ):
                wqk_t = [wp.tile([KC, 2 * KC], fr, tag=f"wqk{k}", name=f"wqk{k}")
                         for k in range(NDC)]
                wv_t = [wp.tile([KC, 2 * KC], fr, tag=f"wv{k}", name=f"wv{k}")
                        for k in range(NDC)]
                for k in range(NDC):
                    nc.sync.dma_start(out=wqk_t[k][:], in_=wqk[k])
                    nc.sync.dma_start(out=wv_t[k][:], in_=wv[k])

                for sblk in range(NSB):
                    for bb in range(B):
                        xs = []
                        for k in range(NDC):
                            xtl = xp.tile([KC, SB], fr, tag="xt")
                            nc.sync.dma_start(out=xtl[:], in_=xt[bb, sblk, k])
                            xs.append(xtl)
                        # m-chunk 0 -> QQ[bb], 1 -> KK[bb]
                        for m in range(2):
                            ps = pp.tile([KC, SB], f32, tag="ps_qk", bufs=4)
                            for k in range(NDC):
                                nc.tensor.matmul(
                                    ps[:],
                                    wqk_t[k][:, m * KC:(m + 1) * KC],
                                    xs[k][:],
                                    start=(k == 0),
                                    stop=(k == NDC - 1),
                                )
                            dest = (QQ if m == 0 else KK)[bb]
                            nc.vector.tensor_scalar_add(
                                dest[:, sblk * SB:(sblk + 1) * SB], ps[:],
                                bqk_t[m][:],
                            )
                        # V natural: lhsT = x^T chunk; rhs = Wv (zero-padded
                        # to N=256 so f32r streams at full rate)
                        for sc in range(SB // KC):
                            ps = pp.tile([KC, 2 * KC], f32, tag="ps_v", bufs=4)
                            for k in range(NDC):
                                nc.tensor.matmul(
                                    ps[:],
                                    xs[k][:, sc * KC:(sc + 1) * KC],
                                    wv_t[k][:],
                                    start=(k == 0),
                                    stop=(k == NDC - 1),
                                )
                            kcg = sblk * (SB // KC) + sc
                            for hh in range(2):
                                nc.vector.tensor_add(
                                    V[2 * bb + hh][:, kcg * (DH + 1):
                                                   kcg * (DH + 1) + DH],
                                    ps[:, hh * DH:(hh + 1) * DH],
                                    bv_bc[:, hh * DH:(hh + 1) * DH],
                                )

            # ---- phase 2: attention --------------------------------------
            with (
                tc.tile_pool(name="pbuf", bufs=1) as pb,
                tc.tile_pool(name="psum_att", bufs=1, space="PSUM") as pa,
            ):
                for qblk in range(NSB):
                    nkc = 4 * (qblk + 1)  # causal: k-chunks 0..nkc-1
                    for p in range(B):
                        P = [
                            [
                                pb.tile([KC, SB], fr, tag=f"P{hh}_{kc}",
                                        name=f"P{hh}{kc}_{p}_{qblk}")
                                for kc in range(nkc)
                            ]
                            for hh in range(2)
                        ]
                        for kc in range(nkc):
                            for hh in range(2):  # row-tiled head pair
                                r0 = hh * DH
                                ps = pa.tile([KC, SB], f32, tag=f"ps_s{hh}", bufs=3)
                                nc.tensor.matmul(
                                    ps[:],
                                    KK[p][r0:r0 + DH, kc * KC:(kc + 1) * KC],
                                    QQ[p][r0:r0 + DH, qblk * SB:(qblk + 1) * SB],
                                    start=True,
                                    stop=True,
                                )
                                nc.scalar.activation(
                                    P[hh][kc][:],
                                    ps[:],
                                    mybir.ActivationFunctionType.Exp,
                                    scale=1.0 / float(np.sqrt(DH)),
                                )
                                d = kc - 4 * qblk
                                if d >= 0:  # diagonal chunk: zero where k > q
                                    nc.gpsimd.affine_select(
                                        out=P[hh][kc][:],
                                        in_=P[hh][kc][:],
                                        pattern=[[1, SB]],
                                        compare_op=mybir.AluOpType.is_ge,
                                        fill=0.0,
                                        base=-KC * d,
                                        channel_multiplier=-1,
                                    )
                        for hh in range(2):
                            po = pa.tile([DH + 1, SB], f32, tag="ps_av", bufs=2)
                            for kc in range(nkc):
                                nc.tensor.matmul(
                                    po[:],
                                    V[2 * p + hh][:, kc * (DH + 1):
                                                  (kc + 1) * (DH + 1)],
                                    P[hh][kc][:],
                                    start=(kc == 0),
                                    stop=(kc == nkc - 1),
                                )
                            # free the psum bank immediately; normalize later
                            avst = ep.tile([DH + 1, SB], f32, tag="avst", bufs=4)
                            nc.scalar.copy(avst[:], po[:])
                            rden = ep.tile([1, SB], f32, tag="rden", bufs=2)
                            nc.vector.reciprocal(rden[:], avst[DH:DH + 1, :])
                            rden_bc = ep.tile([DH, SB], f32, tag="rden_bc", bufs=2)
                            nc.gpsimd.partition_broadcast(
                                out_ap=rden_bc[:], in_ap=rden[:]
                            )
                            r0 = hh * DH
                            nc.vector.tensor_mul(
                                O[p][r0:r0 + DH, qblk * SB:(qblk + 1) * SB],
                                avst[0:DH, :],
                                rden_bc[:],
                            )
                        # stage this (batch, quarter) block for the AllToAll
                        nc.sync.dma_start(
                            out=a2a_in[4 * p + qblk],
                            in_=O[p][:, qblk * SB:(qblk + 1) * SB],
                        )

            # ---- phase 3: head exchange + output projection --------------
            nc.gpsimd.collective_compute(
                "AllToAll",
                mybir.AluOpType.bypass,
                replica_groups=[[0, 1, 2, 3, 4, 5, 6, 7]],
                ins=[a2a_in[:]],
                outs=[a2a_out[:]],
            )
            with (
                tc.tile_pool(name="wout_pool", bufs=1) as wop,
                tc.tile_pool(name="recv", bufs=1) as rp,
                tc.tile_pool(name="psum_out", bufs=1, space="PSUM") as pu,
            ):
                wout_t = [wop.tile([KC, D], fr, tag=f"wo{k}", name=f"wo{k}")
                          for k in range(NDC)]
                for k in range(NDC):
                    nc.sync.dma_start(out=wout_t[k][:], in_=wout[k])
                # a2a_out block i = heads (2i, 2i+1) of my batch for my
                # quarter -> flat [1024, 512] = attnout^T in global head order
                recv = [rp.tile([KC, SB], fr, tag=f"rc{k}", name=f"rc{k}")
                        for k in range(NDC)]
                for k in range(NDC):
                    nc.sync.dma_start(out=recv[k][:], in_=a2a_out[k])
                for sc in range(SB // KC):
                    for nb in range(D // SB):
                        ps = pu.tile([KC, SB], f32, tag="ps_o", bufs=4)
                        for k in range(NDC):
                            nc.tensor.matmul(
                                ps[:],
                                recv[k][:, sc * KC:(sc + 1) * KC],
                                wout_t[k][:, nb * SB:(nb + 1) * SB],
                                start=(k == 0),
                                stop=(k == NDC - 1),
                            )
                        ot = ep.tile([KC, SB], f32, tag="osb", bufs=4)
                        nc.vector.tensor_add(
                            ot[:], ps[:], bo_bc[:, nb * SB:(nb + 1) * SB]
                        )
                        nc.sync.dma_start(
                            out=out_ext[sc * KC:(sc + 1) * KC,
                                        nb * SB:(nb + 1) * SB],
                            in_=ot[:],
                        )

    nc.compile()
    return nc


def _get_program():
    global _compiled
    if _compiled is None:
        _compiled = _build()
    return _compiled


def _shard_inputs(x, Wqkv, bqkv, Wout, bout):
    """Build the 8 per-core input maps (all host-side numpy)."""
    x = np.ascontiguousarray(x, dtype=np.float32)
    Wqkv = np.asarray(Wqkv, dtype=np.float32)
    bqkv = np.asarray(bqkv, dtype=np.float32)
    Wout = np.asarray(Wout, dtype=np.float32)
    bout = np.ascontiguousarray(np.asarray(bout, dtype=np.float32))

    Wq = Wqkv[:, 0 * D:1 * D]
    Wk = Wqkv[:, 1 * D:2 * D]
    Wv_full = Wqkv[:, 2 * D:3 * D]
    bq = bqkv[0 * D:1 * D]
    bk = bqkv[1 * D:2 * D]
    bv_full = bqkv[2 * D:3 * D]

    # shared across all cores
    xt = np.ascontiguousarray(
        x.transpose(0, 2, 1)                      # [B, D, S]
         .reshape(B, D, NSB, SB).transpose(0, 2, 1, 3)
         .reshape(B, NSB, NDC, KC, SB)
    )
    wout_b = np.ascontiguousarray(Wout.reshape(NDC, KC, D))
    vones = np.ones((KC, NKC), dtype=np.float32)

    in_maps = []
    for c in range(NCORES):
        ha, hb = 2 * c, 2 * c + 1
        wqk_c = np.ascontiguousarray(np.concatenate(
            [Wq[:, ha * DH:(ha + 1) * DH], Wq[:, hb * DH:(hb + 1) * DH],
             Wk[:, ha * DH:(ha + 1) * DH], Wk[:, hb * DH:(hb + 1) * DH]],
            axis=1).reshape(NDC, KC, 2 * KC))
        bqk_c = np.ascontiguousarray(np.concatenate(
            [bq[ha * DH:(ha + 1) * DH], bq[hb * DH:(hb + 1) * DH],
             bk[ha * DH:(ha + 1) * DH], bk[hb * DH:(hb + 1) * DH]]))
        # Wv zero-padded to 256 columns so the V matmul moving dim is 256
        wv_c = np.zeros((D, 2 * KC), dtype=np.float32)
        wv_c[:, 0:DH] = Wv_full[:, ha * DH:(ha + 1) * DH]
        wv_c[:, DH:2 * DH] = Wv_full[:, hb * DH:(hb + 1) * DH]
        wv_c = np.ascontiguousarray(wv_c.reshape(NDC, KC, 2 * KC))
        bv_c = np.ascontiguousarray(np.concatenate(
            [bv_full[ha * DH:(ha + 1) * DH], bv_full[hb * DH:(hb + 1) * DH]]))
        in_maps.append({
            "xt": xt, "wqk": wqk_c, "wv": wv_c, "wout": wout_b,
            "bqk": bqk_c, "bv": bv_c, "bo": bout, "vones": vones,
        })
    return in_maps


def run(inputs, trace=False, trace_kwargs=None):
    nc = _get_program()
    in_maps = _shard_inputs(**inputs)
    res = run_bass_kernel_spmd(
        nc, in_maps, list(range(NCORES)), trace=trace,
        **(trace_kwargs or {}),
    )
    out = np.empty((B, S, D), dtype=np.float32)
    for c in range(NCORES):
        b = c // 4
        r0 = SB * (c % 4)
        out[b, r0:r0 + SB, :] = res.results[c]["out"]
    return out, res


def kernel(**inputs):
    out, _ = run(inputs)
    return out


# revision 10
# speedup vs baseline: 1.5688x; 1.4735x over previous
"""Causal multi-head attention (B=2, S=2048, D=1024, H=16) on 8 trn2 cores.

Sharding: core c handles heads {2c, 2c+1} of BOTH batches (4 (b,h) pairs).
Per core:
  - project host-pretransposed x_b^T [D, S] (both batches) through the
    core's Wqkv column slice into Q^T/K^T head-pair tiles and V (natural
    layout, with a fused ones-column that makes the AV matmul emit softmax
    denominators),
  - causal attention per (batch, head) in transposed layout: scores^T =
    K Q^T chunks (PE row-tiled head pairs), exp on ScalarE, causal diagonal
    masks via gpsimd affine_select, A^T V on PE,
  - one 8-wide AllToAll redistributes head outputs so core c holds ALL 16
    heads of batch c//4 for sequence quarter c%4,
  - local projection through the full Wout emits final rows
    512*(c%4) .. +512 of batch c//4.
Host assembles the 8 [512, 1024] shards into (2, 2048, 1024).

Matmuls run in float32r (TF32-like single-pass PE mode, ~1e-3 rel err,
4x faster than true fp32). The PE rounds f32r inputs internally, so DRAM
inputs are declared float32r and DMA'd with the fast HW-DGE path with no
pre-rounding. Set _USE_F32R = False for full fp32.
"""

import sys

for _p in ("/opt/trn_rl_repo", "/opt/pypackages"):
    if _p not in sys.path:
        sys.path.insert(0, _p)

import numpy as np

import concourse.bass as bass
import concourse.mybir as mybir
import concourse.tile as tile
from concourse import bacc
from concourse.bass_utils import run_bass_kernel_spmd

B = 2
S = 2048
D = 1024
H = 16
DH = 64
NCORES = 8
SB = 512           # q block (matmul moving dim)
KC = 128           # k chunk (contraction tile)
NSB = S // SB      # 4 q-blocks
NKC = S // KC      # 16 k-chunks
NDC = D // KC      # 8 contraction chunks for the projections

_USE_F32R = True

_compiled = None


def _build():
    f32 = mybir.dt.float32
    fr = mybir.dt.float32r if _USE_F32R else f32
    nc = bacc.Bacc(None, target_bir_lowering=False)

    # host-blocked inputs: every [128, N] tile is contiguous in DRAM.
    # Matmul inputs are declared float32r: same 4-byte data, PE rounds
    # internally, and plain (non-casting) sync DMA is allowed.
    xt = nc.declare_dram_parameter("xt", [B, NSB, NDC, KC, SB], fr, isOutput=False)
    wqk = nc.declare_dram_parameter("wqk", [NDC, KC, 2 * KC], fr, isOutput=False)
    wv = nc.declare_dram_parameter("wv", [NDC, KC, 2 * KC], fr, isOutput=False)
    wout = nc.declare_dram_parameter("wout", [NDC, KC, D], fr, isOutput=False)
    bqk = nc.declare_dram_parameter("bqk", [2 * KC], f32, isOutput=False)
    bv = nc.declare_dram_parameter("bv", [2 * DH], f32, isOutput=False)
    bo = nc.declare_dram_parameter("bo", [D], f32, isOutput=False)
    vones = nc.declare_dram_parameter("vones", [KC, NKC], fr, isOutput=False)
    out_ext = nc.declare_dram_parameter("out", [SB, D], f32, isOutput=True)

    # AllToAll staging: block t -> core t gets my heads of batch t//4 for
    # s-quarter t%4.
    a2a_in = nc.dram_tensor("a2a_in", [NCORES, KC, SB], fr)
    a2a_out = nc.dram_tensor("a2a_out", [NCORES, KC, SB], fr)

    with tile.TileContext(nc) as tc:
        with (
            tc.tile_pool(name="qkv", bufs=1) as qkvp,
            tc.tile_pool(name="obuf", bufs=1) as op,
            tc.tile_pool(name="misc", bufs=1) as mp,
            tc.tile_pool(name="evict", bufs=1) as ep,
        ):
            # ---- small constants -----------------------------------------
            bqk_t = [mp.tile([KC, 1], f32, tag=f"bqk{m}", name=f"bqk{m}")
                     for m in range(2)]
            for m in range(2):
                nc.sync.dma_start(
                    out=bqk_t[m][:],
                    in_=bqk[m * KC:(m + 1) * KC].rearrange("(p o) -> p o", o=1),
                )
            bv_row = mp.tile([1, 2 * DH], f32, tag="bv_row")
            nc.sync.dma_start(out=bv_row[:], in_=bv.rearrange("(o f) -> o f", o=1))
            bv_bc = mp.tile([KC, 2 * DH], f32, tag="bv_bc")
            nc.gpsimd.partition_broadcast(out_ap=bv_bc[:], in_ap=bv_row[:])
            bo_row = mp.tile([1, D], f32, tag="bo_row")
            nc.sync.dma_start(out=bo_row[:], in_=bo.rearrange("(o f) -> o f", o=1))
            bo_bc = mp.tile([KC, D], f32, tag="bo_bc")
            nc.gpsimd.partition_broadcast(out_ap=bo_bc[:], in_ap=bo_row[:])

            # ---- persistent activations ----------------------------------
            # pair p = batch p with heads (2c, 2c+1).
            # QQ[p]: rows 0:64 = Q^T of head 2c, rows 64:128 = head 2c+1
            QQ = [qkvp.tile([KC, S], fr, tag=f"QQ{p}", name=f"QQ{p}") for p in range(2)]
            KK = [qkvp.tile([KC, S], fr, tag=f"KK{p}", name=f"KK{p}") for p in range(2)]
            # V[2p+hh]: [128, 16*65]; chunk kc at cols kc*65..+64; col 64: 1.0
            V = [qkvp.tile([KC, NKC * (DH + 1)], fr, tag=f"V{v}", name=f"V{v}")
                 for v in range(4)]
            for v in range(4):
                vv = V[v][:].rearrange("p (k c) -> p k c", c=DH + 1)
                nc.sync.dma_start(out=vv[:, :, DH], in_=vones[:, :])
            # O[p]: rows 0:64 = head 2c out^T (normalized), 64:128 = head 2c+1
            O = [op.tile([KC, S], fr, tag=f"O{p}", name=f"O{p}") for p in range(2)]

            # ---- phase 1: projections ------------------------------------
            with (
                tc.tile_pool(name="pjw", bufs=1) as wp,
                tc.tile_pool(name="xbuf", bufs=24) as xp,
                tc.tile_pool(name="psum_proj", bufs=1, space="PSUM") as pp,
            ):
                wqk_t = [wp.tile([KC, 2 * KC], fr, tag=f"wqk{k}", name=f"wqk{k}")
                         for k in range(NDC)]
                wv_t = [wp.tile([KC, 2 * KC], fr, tag=f"wv{k}", name=f"wv{k}")
                        for k in range(NDC)]
                for k in range(NDC):
                    nc.sync.dma_start(out=wqk_t[k][:], in_=wqk[k])
                    nc.sync.dma_start(out=wv_t[k][:], in_=wv[k])

                for sblk in range(NSB):
                    for bb in range(B):
                        xs = []
                        for k in range(NDC):
                            xtl = xp.tile([KC, SB], fr, tag="xt")
                            nc.sync.dma_start(out=xtl[:], in_=xt[bb, sblk, k])
                            xs.append(xtl)
                        # m-chunk 0 -> QQ[bb], 1 -> KK[bb]
                        for m in range(2):
                            ps = pp.tile([KC, SB], f32, tag="ps_qk", bufs=4)
                            for k in range(NDC):
                                nc.tensor.matmul(
                                    ps[:],
                                    wqk_t[k][:, m * KC:(m + 1) * KC],
                                    xs[k][:],
                                    start=(k == 0),
                                    stop=(k == NDC - 1),
                                )
                            dest = (QQ if m == 0 else KK)[bb]
                            nc.vector.tensor_scalar_add(
                                dest[:, sblk * SB:(sblk + 1) * SB], ps[:],
                                bqk_t[m][:],
                            )
                        # V natural: lhsT = x^T chunk; rhs = Wv (zero-padded
                        # to N=256 so f32r streams at full rate)
                        for sc in range(SB // KC):
                            ps = pp.tile([KC, 2 * KC], f32, tag="ps_v", bufs=4)
                            for k in range(NDC):
                                nc.tensor.matmul(
                                    ps[:],
                                    xs[k][:, sc * KC:(sc + 1) * KC],
                                    wv_t[k][:],
                                    start=(k == 0),
                                    stop=(k == NDC - 1),
                                )
                            kcg = sblk * (SB // KC) + sc
                            for hh in range(2):
                                nc.vector.tensor_add(
                                    V[2 * bb + hh][:, kcg * (DH + 1):
                                                   kcg * (DH + 1) + DH],
                                    ps[:, hh * DH:(hh + 1) * DH],
                                    bv_bc[:, hh * DH:(hh + 1) * DH],
                                )

            # ---- phase 2: attention --------------------------------------
            with (
                tc.tile_pool(name="pbuf", bufs=1) as pb,
                tc.tile_pool(name="psum_att", bufs=1, space="PSUM") as pa,
            ):
                for qblk in range(NSB):
                    nkc = 4 * (qblk + 1)  # causal: k-chunks 0..nkc-1
                    for p in range(B):
                        # P[kc]: [128, 1024]; cols hh*512.. hold head hh
                        P = [
                            pb.tile([KC, 2 * SB], fr, tag=f"P{kc}",
                                    name=f"P{kc}_{p}_{qblk}")
                            for kc in range(nkc)
                        ]
                        for kc in range(nkc):
                            d = kc - 4 * qblk
                            # causal: columns < 128*d are fully masked; skip
                            # them in the matmul/exp where the speed holds up
                            c0 = min(KC * max(d, 0), 2 * KC)
                            ps = pa.tile([KC, 2 * SB], f32, tag="ps_s", bufs=3)
                            for hh in range(2):  # row-tiled head pair
                                r0 = hh * DH
                                nc.tensor.matmul(
                                    ps[:, hh * SB + c0:(hh + 1) * SB],
                                    KK[p][r0:r0 + DH, kc * KC:(kc + 1) * KC],
                                    QQ[p][r0:r0 + DH,
                                         qblk * SB + c0:(qblk + 1) * SB],
                                    start=True,
                                    stop=True,
                                )
                            ps3 = ps[:].rearrange("p (h f) -> p h f", h=2)
                            pd3 = P[kc][:].rearrange("p (h f) -> p h f", h=2)
                            e0 = KC * max(d, 0)
                            nc.scalar.activation(
                                pd3[:, :, e0:SB],
                                ps3[:, :, e0:SB],
                                mybir.ActivationFunctionType.Exp,
                                scale=1.0 / float(np.sqrt(DH)),
                            )
                            if d >= 0:  # diagonal chunk: zero where k > q
                                nc.gpsimd.affine_select(
                                    out=pd3[:, :, :],
                                    in_=pd3[:, :, :],
                                    pattern=[[0, 2], [1, SB]],
                                    compare_op=mybir.AluOpType.is_ge,
                                    fill=0.0,
                                    base=-KC * d,
                                    channel_multiplier=-1,
                                )
                        for hh in range(2):
                            po = pa.tile([DH + 1, SB], f32, tag="ps_av", bufs=2)
                            for kc in range(nkc):
                                d = kc - 4 * qblk
                                c0 = min(KC * max(d, 0), 2 * KC)
                                nc.tensor.matmul(
                                    po[:, c0:SB],
                                    V[2 * p + hh][:, kc * (DH + 1):
                                                  (kc + 1) * (DH + 1)],
                                    P[kc][:, hh * SB + c0:(hh + 1) * SB],
                                    start=(kc == 0),
                                    stop=(kc == nkc - 1),
                                )
                            # free the psum bank immediately; normalize later
                            avst = ep.tile([DH + 1, SB], f32, tag="avst", bufs=4)
                            nc.vector.tensor_copy(avst[:], po[:])
                            rden = ep.tile([1, SB], f32, tag="rden", bufs=2)
                            nc.vector.reciprocal(rden[:], avst[DH:DH + 1, :])
                            rden_bc = ep.tile([DH, SB], f32, tag="rden_bc", bufs=2)
                            nc.gpsimd.partition_broadcast(
                                out_ap=rden_bc[:], in_ap=rden[:]
                            )
                            r0 = hh * DH
                            nc.vector.tensor_mul(
                                O[p][r0:r0 + DH, qblk * SB:(qblk + 1) * SB],
                                avst[0:DH, :],
                                rden_bc[:],
                            )
                        # stage this (batch, quarter) block for the AllToAll
                        nc.sync.dma_start(
                            out=a2a_in[4 * p + qblk],
                            in_=O[p][:, qblk * SB:(qblk + 1) * SB],
                        )

            # ---- phase 3: head exchange + output projection --------------
            nc.gpsimd.collective_compute(
                "AllToAll",
                mybir.AluOpType.bypass,
                replica_groups=[[0, 1, 2, 3, 4, 5, 6, 7]],
                ins=[a2a_in[:]],
                outs=[a2a_out[:]],
            )
            with (
                tc.tile_pool(name="wout_pool", bufs=1) as wop,
                tc.tile_pool(name="recv", bufs=1) as rp,
                tc.tile_pool(name="psum_out", bufs=1, space="PSUM") as pu,
            ):
                wout_t = [wop.tile([KC, D], fr, tag=f"wo{k}", name=f"wo{k}")
                          for k in range(NDC)]
                for k in range(NDC):
                    nc.sync.dma_start(out=wout_t[k][:], in_=wout[k])
                # a2a_out block i = heads (2i, 2i+1) of my batch for my
                # quarter -> flat [1024, 512] = attnout^T in global head order
                recv = [rp.tile([KC, SB], fr, tag=f"rc{k}", name=f"rc{k}")
                        for k in range(NDC)]
                for k in range(NDC):
                    nc.sync.dma_start(out=recv[k][:], in_=a2a_out[k])
                for sc in range(SB // KC):
                    for nb in range(D // SB):
                        ps = pu.tile([KC, SB], f32, tag="ps_o", bufs=4)
                        for k in range(NDC):
                            nc.tensor.matmul(
                                ps[:],
                                recv[k][:, sc * KC:(sc + 1) * KC],
                                wout_t[k][:, nb * SB:(nb + 1) * SB],
                                start=(k == 0),
                                stop=(k == NDC - 1),
                            )
                        ot = ep.tile([KC, SB], f32, tag="osb", bufs=4)
                        nc.vector.tensor_add(
                            ot[:], ps[:], bo_bc[:, nb * SB:(nb + 1) * SB]
                        )
                        nc.sync.dma_start(
                            out=out_ext[sc * KC:(sc + 1) * KC,
                                        nb * SB:(nb + 1) * SB],
                            in_=ot[:],
                        )

    nc.compile()
    return nc


def _get_program():
    global _compiled
    if _compiled is None:
        _compiled = _build()
    return _compiled


def _shard_inputs(x, Wqkv, bqkv, Wout, bout):
    """Build the 8 per-core input maps (all host-side numpy)."""
    x = np.ascontiguousarray(x, dtype=np.float32)
    Wqkv = np.asarray(Wqkv, dtype=np.float32)
    bqkv = np.asarray(bqkv, dtype=np.float32)
    Wout = np.asarray(Wout, dtype=np.float32)
    bout = np.ascontiguousarray(np.asarray(bout, dtype=np.float32))

    Wq = Wqkv[:, 0 * D:1 * D]
    Wk = Wqkv[:, 1 * D:2 * D]
    Wv_full = Wqkv[:, 2 * D:3 * D]
    bq = bqkv[0 * D:1 * D]
    bk = bqkv[1 * D:2 * D]
    bv_full = bqkv[2 * D:3 * D]

    # shared across all cores
    xt = np.ascontiguousarray(
        x.transpose(0, 2, 1)                      # [B, D, S]
         .reshape(B, D, NSB, SB).transpose(0, 2, 1, 3)
         .reshape(B, NSB, NDC, KC, SB)
    )
    wout_b = np.ascontiguousarray(Wout.reshape(NDC, KC, D))
    vones = np.ones((KC, NKC), dtype=np.float32)

    in_maps = []
    for c in range(NCORES):
        ha, hb = 2 * c, 2 * c + 1
        wqk_c = np.ascontiguousarray(np.concatenate(
            [Wq[:, ha * DH:(ha + 1) * DH], Wq[:, hb * DH:(hb + 1) * DH],
             Wk[:, ha * DH:(ha + 1) * DH], Wk[:, hb * DH:(hb + 1) * DH]],
            axis=1).reshape(NDC, KC, 2 * KC))
        bqk_c = np.ascontiguousarray(np.concatenate(
            [bq[ha * DH:(ha + 1) * DH], bq[hb * DH:(hb + 1) * DH],
             bk[ha * DH:(ha + 1) * DH], bk[hb * DH:(hb + 1) * DH]]))
        # Wv zero-padded to 256 columns so the V matmul moving dim is 256
        wv_c = np.zeros((D, 2 * KC), dtype=np.float32)
        wv_c[:, 0:DH] = Wv_full[:, ha * DH:(ha + 1) * DH]
        wv_c[:, DH:2 * DH] = Wv_full[:, hb * DH:(hb + 1) * DH]
        wv_c = np.ascontiguousarray(wv_c.reshape(NDC, KC, 2 * KC))
        bv_c = np.ascontiguousarray(np.concatenate(
            [bv_full[ha * DH:(ha + 1) * DH], bv_full[hb * DH:(hb + 1) * DH]]))
        in_maps.append({
            "xt": xt, "wqk": wqk_c, "wv": wv_c, "wout": wout_b,
            "bqk": bqk_c, "bv": bv_c, "bo": bout, "vones": vones,
        })
    return in_maps


def run(inputs, trace=False, trace_kwargs=None):
    nc = _get_program()
    in_maps = _shard_inputs(**inputs)
    res = run_bass_kernel_spmd(
        nc, in_maps, list(range(NCORES)), trace=trace,
        **(trace_kwargs or {}),
    )
    out = np.empty((B, S, D), dtype=np.float32)
    for c in range(NCORES):
        b = c // 4
        r0 = SB * (c % 4)
        out[b, r0:r0 + SB, :] = res.results[c]["out"]
    return out, res


def kernel(**inputs):
    out, _ = run(inputs)
    return out


# revision 12
# speedup vs baseline: 1.6622x; 1.0595x over previous
"""Causal multi-head attention (B=2, S=2048, D=1024, H=16) on 8 trn2 cores.

Sharding: core c handles heads {2c, 2c+1} of BOTH batches (4 (b,h) pairs).
Per core:
  - project host-pretransposed x_b^T [D, S] (both batches) through the
    core's Wqkv column slice into Q^T/K^T head-pair tiles and V (natural
    layout, with a fused ones-column that makes the AV matmul emit softmax
    denominators),
  - causal attention per (batch, head) in transposed layout: scores^T =
    K Q^T chunks (PE row-tiled head pairs), exp on ScalarE, causal diagonal
    masks via gpsimd affine_select, A^T V on PE,
  - one 8-wide AllToAll redistributes head outputs so core c holds ALL 16
    heads of batch c//4 for sequence quarter c%4,
  - local projection through the full Wout emits final rows
    512*(c%4) .. +512 of batch c//4.
Host assembles the 8 [512, 1024] shards into (2, 2048, 1024).

Matmuls run in float32r (TF32-like single-pass PE mode, ~1e-3 rel err,
4x faster than true fp32). The PE rounds f32r inputs internally, so DRAM
inputs are declared float32r and DMA'd with the fast HW-DGE path with no
pre-rounding. Set _USE_F32R = False for full fp32.
"""

import sys

for _p in ("/opt/trn_rl_repo", "/opt/pypackages"):
    if _p not in sys.path:
        sys.path.insert(0, _p)

import numpy as np

import concourse.bass as bass
import concourse.mybir as mybir
import concourse.tile as tile
from concourse import bacc
from concourse.bass_utils import run_bass_kernel_spmd

B = 2
S = 2048
D = 1024
H = 16
DH = 64
NCORES = 8
SB = 512           # q block (matmul moving dim)
KC = 128           # k chunk (contraction tile)
NSB = S // SB      # 4 q-blocks
NKC = S // KC      # 16 k-chunks
NDC = D // KC      # 8 contraction chunks for the projections

_USE_F32R = True

_compiled = None


def _build():
    f32 = mybir.dt.float32
    fr = mybir.dt.float32r if _USE_F32R else f32
    nc = bacc.Bacc(None, target_bir_lowering=False)

    # host-blocked inputs: every [128, N] tile is contiguous in DRAM.
    # Matmul inputs are declared float32r: same 4-byte data, PE rounds
    # internally, and plain (non-casting) sync DMA is allowed.
    xt = nc.declare_dram_parameter("xt", [B, NSB, NDC, KC, SB], fr, isOutput=False)
    wqk = nc.declare_dram_parameter("wqk", [NDC, KC, 2 * KC], fr, isOutput=False)
    wv = nc.declare_dram_parameter("wv", [NDC, KC, 2 * KC], fr, isOutput=False)
    wout = nc.declare_dram_parameter("wout", [NDC, KC, D], fr, isOutput=False)
    bqk = nc.declare_dram_parameter("bqk", [2 * KC], f32, isOutput=False)
    bv = nc.declare_dram_parameter("bv", [2 * DH], f32, isOutput=False)
    bo = nc.declare_dram_parameter("bo", [D], f32, isOutput=False)
    vones = nc.declare_dram_parameter("vones", [KC, NKC], fr, isOutput=False)
    out_ext = nc.declare_dram_parameter("out", [SB, D], f32, isOutput=True)

    # AllToAll staging: block t -> core t gets my heads of batch t//4 for
    # s-quarter t%4.
    a2a_in = nc.dram_tensor("a2a_in", [NCORES, KC, SB], fr)
    a2a_out = nc.dram_tensor("a2a_out", [NCORES, KC, SB], fr)

    with tile.TileContext(nc) as tc:
        with (
            tc.tile_pool(name="qkv", bufs=1) as qkvp,
            tc.tile_pool(name="obuf", bufs=1) as op,
            tc.tile_pool(name="misc", bufs=1) as mp,
            tc.tile_pool(name="evict", bufs=1) as ep,
        ):
            # ---- small constants -----------------------------------------
            bqk_t = [mp.tile([KC, 1], f32, tag=f"bqk{m}", name=f"bqk{m}")
                     for m in range(2)]
            for m in range(2):
                nc.sync.dma_start(
                    out=bqk_t[m][:],
                    in_=bqk[m * KC:(m + 1) * KC].rearrange("(p o) -> p o", o=1),
                )
            bv_row = mp.tile([1, 2 * DH], f32, tag="bv_row")
            nc.sync.dma_start(out=bv_row[:], in_=bv.rearrange("(o f) -> o f", o=1))
            bv_bc = mp.tile([KC, 2 * DH], f32, tag="bv_bc")
            nc.gpsimd.partition_broadcast(out_ap=bv_bc[:], in_ap=bv_row[:])
            bo_row = mp.tile([1, D], f32, tag="bo_row")
            nc.sync.dma_start(out=bo_row[:], in_=bo.rearrange("(o f) -> o f", o=1))
            bo_bc = mp.tile([KC, D], f32, tag="bo_bc")
            nc.gpsimd.partition_broadcast(out_ap=bo_bc[:], in_ap=bo_row[:])

            # ---- persistent activations ----------------------------------
            # pair p = batch p with heads (2c, 2c+1).
            # QQ[p]: rows 0:64 = Q^T of head 2c, rows 64:128 = head 2c+1
            QQ = [qkvp.tile([KC, S], fr, tag=f"QQ{p}", name=f"QQ{p}") for p in range(2)]
            KK = [qkvp.tile([KC, S], fr, tag=f"KK{p}", name=f"KK{p}") for p in range(2)]
            # V[2p+hh]: [128, 16*65]; chunk kc at cols kc*65..+64; col 64: 1.0
            V = [qkvp.tile([KC, NKC * (DH + 1)], fr, tag=f"V{v}", name=f"V{v}")
                 for v in range(4)]
            vones_sb = mp.tile([KC, NKC], fr, tag="vones_sb")
            nc.sync.dma_start(out=vones_sb[:], in_=vones[:])
            for v in range(4):
                vv = V[v][:].rearrange("p (k c) -> p k c", c=DH + 1)
                nc.vector.tensor_copy(vv[:, :, DH], vones_sb[:])
            # O[p]: rows 0:64 = head 2c out^T (normalized), 64:128 = head 2c+1
            O = [op.tile([KC, S], fr, tag=f"O{p}", name=f"O{p}") for p in range(2)]

            # ---- phase 1: projections ------------------------------------
            with (
                tc.tile_pool(name="pjw", bufs=1) as wp,
                tc.tile_pool(name="xbuf", bufs=24) as xp,
                tc.tile_pool(name="psum_proj", bufs=1, space="PSUM") as pp,
            ):
                wqk_t = [wp.tile([KC, 2 * KC], fr, tag=f"wqk{k}", name=f"wqk{k}")
                         for k in range(NDC)]
                wv_t = [wp.tile([KC, 2 * KC], fr, tag=f"wv{k}", name=f"wv{k}")
                        for k in range(NDC)]
                for k in range(NDC):
                    nc.sync.dma_start(out=wqk_t[k][:], in_=wqk[k])
                    nc.sync.dma_start(out=wv_t[k][:], in_=wv[k])

                for sblk in range(NSB):
                    for bb in range(B):
                        xs = []
                        for k in range(NDC):
                            xtl = xp.tile([KC, SB], fr, tag="xt")
                            nc.sync.dma_start(out=xtl[:], in_=xt[bb, sblk, k])
                            xs.append(xtl)
                        # m-chunk 0 -> QQ[bb], 1 -> KK[bb]
                        for m in range(2):
                            ps = pp.tile([KC, SB], f32, tag="ps_qk", bufs=4)
                            for k in range(NDC):
                                nc.tensor.matmul(
                                    ps[:],
                                    wqk_t[k][:, m * KC:(m + 1) * KC],
                                    xs[k][:],
                                    start=(k == 0),
                                    stop=(k == NDC - 1),
                                )
                            dest = (QQ if m == 0 else KK)[bb]
                            nc.vector.tensor_scalar_add(
                                dest[:, sblk * SB:(sblk + 1) * SB], ps[:],
                                bqk_t[m][:],
                            )
                        # V natural: lhsT = x^T chunk; rhs = Wv (zero-padded
                        # to N=256 so f32r streams at full rate)
                        for sc in range(SB // KC):
                            ps = pp.tile([KC, 2 * KC], f32, tag="ps_v", bufs=4)
                            for k in range(NDC):
                                nc.tensor.matmul(
                                    ps[:],
                                    xs[k][:, sc * KC:(sc + 1) * KC],
                                    wv_t[k][:],
                                    start=(k == 0),
                                    stop=(k == NDC - 1),
                                )
                            kcg = sblk * (SB // KC) + sc
                            for hh in range(2):
                                nc.vector.tensor_add(
                                    V[2 * bb + hh][:, kcg * (DH + 1):
                                                   kcg * (DH + 1) + DH],
                                    ps[:, hh * DH:(hh + 1) * DH],
                                    bv_bc[:, hh * DH:(hh + 1) * DH],
                                )

            # ---- phase 2: attention --------------------------------------
            with (
                tc.tile_pool(name="pbuf", bufs=1) as pb,
                tc.tile_pool(name="psum_att", bufs=1, space="PSUM") as pa,
            ):
                for qblk in range(NSB):
                    nkc = 4 * (qblk + 1)  # causal: k-chunks 0..nkc-1
                    for p in range(B):
                        # P[kc]: [128, 1024]; cols hh*512.. hold head hh
                        P = [
                            pb.tile([KC, 2 * SB], fr, tag=f"P{kc}",
                                    name=f"P{kc}_{p}_{qblk}",
                                    bufs=(2 if kc < 11 else 1))
                            for kc in range(nkc)
                        ]
                        for kc in range(nkc):
                            d = kc - 4 * qblk
                            # causal: columns < 128*d are fully masked; skip
                            # them in the matmul/exp where the speed holds up
                            c0 = min(KC * max(d, 0), 2 * KC)
                            ps = pa.tile([KC, 2 * SB], f32, tag="ps_s", bufs=3)
                            for hh in range(2):  # row-tiled head pair
                                r0 = hh * DH
                                nc.tensor.matmul(
                                    ps[:, hh * SB + c0:(hh + 1) * SB],
                                    KK[p][r0:r0 + DH, kc * KC:(kc + 1) * KC],
                                    QQ[p][r0:r0 + DH,
                                         qblk * SB + c0:(qblk + 1) * SB],
                                    start=True,
                                    stop=True,
                                )
                            ps3 = ps[:].rearrange("p (h f) -> p h f", h=2)
                            pd3 = P[kc][:].rearrange("p (h f) -> p h f", h=2)
                            e0 = KC * max(d, 0)
                            nc.scalar.activation(
                                pd3[:, :, e0:SB],
                                ps3[:, :, e0:SB],
                                mybir.ActivationFunctionType.Exp,
                                scale=1.0 / float(np.sqrt(DH)),
                            )
                            if d >= 0:  # diagonal chunk: zero where k > q
                                nc.gpsimd.affine_select(
                                    out=pd3[:, :, :],
                                    in_=pd3[:, :, :],
                                    pattern=[[0, 2], [1, SB]],
                                    compare_op=mybir.AluOpType.is_ge,
                                    fill=0.0,
                                    base=-KC * d,
                                    channel_multiplier=-1,
                                )
                        for hh in range(2):
                            po = pa.tile([DH + 1, SB], f32, tag="ps_av", bufs=2)
                            for kc in range(nkc):
                                d = kc - 4 * qblk
                                c0 = min(KC * max(d, 0), 2 * KC)
                                nc.tensor.matmul(
                                    po[:, c0:SB],
                                    V[2 * p + hh][:, kc * (DH + 1):
                                                  (kc + 1) * (DH + 1)],
                                    P[kc][:, hh * SB + c0:(hh + 1) * SB],
                                    start=(kc == 0),
                                    stop=(kc == nkc - 1),
                                )
                            # free the psum bank immediately; normalize later
                            avst = ep.tile([DH + 1, SB], f32, tag="avst", bufs=4)
                            nc.vector.tensor_copy(avst[:], po[:])
                            rden = ep.tile([1, SB], f32, tag="rden", bufs=2)
                            nc.vector.reciprocal(rden[:], avst[DH:DH + 1, :])
                            rden_bc = ep.tile([DH, SB], f32, tag="rden_bc", bufs=2)
                            nc.gpsimd.partition_broadcast(
                                out_ap=rden_bc[:], in_ap=rden[:]
                            )
                            r0 = hh * DH
                            nc.vector.tensor_mul(
                                O[p][r0:r0 + DH, qblk * SB:(qblk + 1) * SB],
                                avst[0:DH, :],
                                rden_bc[:],
                            )
                        # stage this (batch, quarter) block for the AllToAll
                        nc.sync.dma_start(
                            out=a2a_in[4 * p + qblk],
                            in_=O[p][:, qblk * SB:(qblk + 1) * SB],
                        )

            # ---- phase 3: head exchange + output projection --------------
            nc.gpsimd.collective_compute(
                "AllToAll",
                mybir.AluOpType.bypass,
                replica_groups=[[0, 1, 2, 3, 4, 5, 6, 7]],
                ins=[a2a_in[:]],
                outs=[a2a_out[:]],
            )
            with (
                tc.tile_pool(name="wout_pool", bufs=1) as wop,
                tc.tile_pool(name="recv", bufs=1) as rp,
                tc.tile_pool(name="psum_out", bufs=1, space="PSUM") as pu,
            ):
                wout_t = [wop.tile([KC, D], fr, tag=f"wo{k}", name=f"wo{k}")
                          for k in range(NDC)]
                for k in range(NDC):
                    nc.sync.dma_start(out=wout_t[k][:], in_=wout[k])
                # a2a_out block i = heads (2i, 2i+1) of my batch for my
                # quarter -> flat [1024, 512] = attnout^T in global head order
                recv = [rp.tile([KC, SB], fr, tag=f"rc{k}", name=f"rc{k}")
                        for k in range(NDC)]
                for k in range(NDC):
                    nc.sync.dma_start(out=recv[k][:], in_=a2a_out[k])
                for sc in range(SB // KC):
                    for nb in range(D // SB):
                        ps = pu.tile([KC, SB], f32, tag="ps_o", bufs=4)
                        for k in range(NDC):
                            nc.tensor.matmul(
                                ps[:],
                                recv[k][:, sc * KC:(sc + 1) * KC],
                                wout_t[k][:, nb * SB:(nb + 1) * SB],
                                start=(k == 0),
                                stop=(k == NDC - 1),
                            )
                        ot = ep.tile([KC, SB], f32, tag="osb", bufs=4)
                        nc.vector.tensor_add(
                            ot[:], ps[:], bo_bc[:, nb * SB:(nb + 1) * SB]
                        )
                        nc.sync.dma_start(
                            out=out_ext[sc * KC:(sc + 1) * KC,
                                        nb * SB:(nb + 1) * SB],
                            in_=ot[:],
                        )

    nc.compile()
    return nc


def _get_program():
    global _compiled
    if _compiled is None:
        _compiled = _build()
    return _compiled


def _shard_inputs(x, Wqkv, bqkv, Wout, bout):
    """Build the 8 per-core input maps (all host-side numpy)."""
    x = np.ascontiguousarray(x, dtype=np.float32)
    Wqkv = np.asarray(Wqkv, dtype=np.float32)
    bqkv = np.asarray(bqkv, dtype=np.float32)
    Wout = np.asarray(Wout, dtype=np.float32)
    bout = np.ascontiguousarray(np.asarray(bout, dtype=np.float32))

    Wq = Wqkv[:, 0 * D:1 * D]
    Wk = Wqkv[:, 1 * D:2 * D]
    Wv_full = Wqkv[:, 2 * D:3 * D]
    bq = bqkv[0 * D:1 * D]
    bk = bqkv[1 * D:2 * D]
    bv_full = bqkv[2 * D:3 * D]

    # shared across all cores
    xt = np.ascontiguousarray(
        x.transpose(0, 2, 1)                      # [B, D, S]
         .reshape(B, D, NSB, SB).transpose(0, 2, 1, 3)
         .reshape(B, NSB, NDC, KC, SB)
    )
    wout_b = np.ascontiguousarray(Wout.reshape(NDC, KC, D))
    vones = np.ones((KC, NKC), dtype=np.float32)

    in_maps = []
    for c in range(NCORES):
        ha, hb = 2 * c, 2 * c + 1
        wqk_c = np.ascontiguousarray(np.concatenate(
            [Wq[:, ha * DH:(ha + 1) * DH], Wq[:, hb * DH:(hb + 1) * DH],
             Wk[:, ha * DH:(ha + 1) * DH], Wk[:, hb * DH:(hb + 1) * DH]],
            axis=1).reshape(NDC, KC, 2 * KC))
        bqk_c = np.ascontiguousarray(np.concatenate(
            [bq[ha * DH:(ha + 1) * DH], bq[hb * DH:(hb + 1) * DH],
             bk[ha * DH:(ha + 1) * DH], bk[hb * DH:(hb + 1) * DH]]))
        # Wv zero-padded to 256 columns so the V matmul moving dim is 256
        wv_c = np.zeros((D, 2 * KC), dtype=np.float32)
        wv_c[:, 0:DH] = Wv_full[:, ha * DH:(ha + 1) * DH]
        wv_c[:, DH:2 * DH] = Wv_full[:, hb * DH:(hb + 1) * DH]
        wv_c = np.ascontiguousarray(wv_c.reshape(NDC, KC, 2 * KC))
        bv_c = np.ascontiguousarray(np.concatenate(
            [bv_full[ha * DH:(ha + 1) * DH], bv_full[hb * DH:(hb + 1) * DH]]))
        in_maps.append({
            "xt": xt, "wqk": wqk_c, "wv": wv_c, "wout": wout_b,
            "bqk": bqk_c, "bv": bv_c, "bo": bout, "vones": vones,
        })
    return in_maps


def run(inputs, trace=False, trace_kwargs=None):
    nc = _get_program()
    in_maps = _shard_inputs(**inputs)
    res = run_bass_kernel_spmd(
        nc, in_maps, list(range(NCORES)), trace=trace,
        **(trace_kwargs or {}),
    )
    out = np.empty((B, S, D), dtype=np.float32)
    for c in range(NCORES):
        b = c // 4
        r0 = SB * (c % 4)
        out[b, r0:r0 + SB, :] = res.results[c]["out"]
    return out, res


def kernel(**inputs):
    out, _ = run(inputs)
    return out


# revision 13
# speedup vs baseline: 1.6671x; 1.0030x over previous
"""Causal multi-head attention (B=2, S=2048, D=1024, H=16) on 8 trn2 cores.

Sharding: core c handles heads {2c, 2c+1} of BOTH batches (4 (b,h) pairs).
Per core:
  - project host-pretransposed x_b^T [D, S] (both batches) through the
    core's Wqkv column slice into Q^T/K^T head-pair tiles and V (natural
    layout, with a fused ones-column that makes the AV matmul emit softmax
    denominators),
  - causal attention per (batch, head) in transposed layout: scores^T =
    K Q^T chunks (PE row-tiled head pairs), exp on ScalarE, causal diagonal
    masks via gpsimd affine_select, A^T V on PE,
  - one 8-wide AllToAll redistributes head outputs so core c holds ALL 16
    heads of batch c//4 for sequence quarter c%4,
  - local projection through the full Wout emits final rows
    512*(c%4) .. +512 of batch c//4.
Host assembles the 8 [512, 1024] shards into (2, 2048, 1024).

Matmuls run in float32r (TF32-like single-pass PE mode, ~1e-3 rel err,
4x faster than true fp32). The PE rounds f32r inputs internally, so DRAM
inputs are declared float32r and DMA'd with the fast HW-DGE path with no
pre-rounding. Set _USE_F32R = False for full fp32.
"""

import sys

for _p in ("/opt/trn_rl_repo", "/opt/pypackages"):
    if _p not in sys.path:
        sys.path.insert(0, _p)

import numpy as np

import concourse.bass as bass
import concourse.mybir as mybir
import concourse.tile as tile
from concourse import bacc
from concourse.bass_utils import run_bass_kernel_spmd

B = 2
S = 2048
D = 1024
H = 16
DH = 64
NCORES = 8
SB = 512           # q block (matmul moving dim)
KC = 128           # k chunk (contraction tile)
NSB = S // SB      # 4 q-blocks
NKC = S // KC      # 16 k-chunks
NDC = D // KC      # 8 contraction chunks for the projections

_USE_F32R = True

_compiled = None


def _build():
    f32 = mybir.dt.float32
    fr = mybir.dt.float32r if _USE_F32R else f32
    nc = bacc.Bacc(None, target_bir_lowering=False)

    # host-blocked inputs: every [128, N] tile is contiguous in DRAM.
    # Matmul inputs are declared float32r: same 4-byte data, PE rounds
    # internally, and plain (non-casting) sync DMA is allowed.
    xt = nc.declare_dram_parameter("xt", [B, NSB, NDC, KC, SB], fr, isOutput=False)
    wqk = nc.declare_dram_parameter("wqk", [NDC, KC, 2 * KC], fr, isOutput=False)
    wv = nc.declare_dram_parameter("wv", [NDC, KC, 2 * KC], fr, isOutput=False)
    wout = nc.declare_dram_parameter("wout", [NDC, KC, D], fr, isOutput=False)
    bqk = nc.declare_dram_parameter("bqk", [2 * KC], f32, isOutput=False)
    bv = nc.declare_dram_parameter("bv", [2 * DH], f32, isOutput=False)
    bo = nc.declare_dram_parameter("bo", [D], f32, isOutput=False)
    vones = nc.declare_dram_parameter("vones", [KC, NKC], fr, isOutput=False)
    out_ext = nc.declare_dram_parameter("out", [SB, D], f32, isOutput=True)

    # AllToAll staging: block t -> core t gets my heads of batch t//4 for
    # s-quarter t%4.
    a2a_in = nc.dram_tensor("a2a_in", [NCORES, KC, SB], fr)
    a2a_out = nc.dram_tensor("a2a_out", [NCORES, KC, SB], fr)

    with tile.TileContext(nc) as tc:
        with (
            tc.tile_pool(name="qkv", bufs=1) as qkvp,
            tc.tile_pool(name="obuf", bufs=1) as op,
            tc.tile_pool(name="misc", bufs=1) as mp,
            tc.tile_pool(name="evict", bufs=1) as ep,
        ):
            # ---- small constants -----------------------------------------
            bqk_t = [mp.tile([KC, 1], f32, tag=f"bqk{m}", name=f"bqk{m}")
                     for m in range(2)]
            for m in range(2):
                nc.sync.dma_start(
                    out=bqk_t[m][:],
                    in_=bqk[m * KC:(m + 1) * KC].rearrange("(p o) -> p o", o=1),
                )
            bv_row = mp.tile([1, 2 * DH], f32, tag="bv_row")
            nc.sync.dma_start(out=bv_row[:], in_=bv.rearrange("(o f) -> o f", o=1))
            bv_bc = mp.tile([KC, 2 * DH], f32, tag="bv_bc")
            nc.gpsimd.partition_broadcast(out_ap=bv_bc[:], in_ap=bv_row[:])
            bo_row = mp.tile([1, D], f32, tag="bo_row")
            nc.sync.dma_start(out=bo_row[:], in_=bo.rearrange("(o f) -> o f", o=1))
            bo_bc = mp.tile([KC, D], f32, tag="bo_bc")
            nc.gpsimd.partition_broadcast(out_ap=bo_bc[:], in_ap=bo_row[:])

            # ---- persistent activations ----------------------------------
            # pair p = batch p with heads (2c, 2c+1).
            # QQ[p]: rows 0:64 = Q^T of head 2c, rows 64:128 = head 2c+1
            QQ = [qkvp.tile([KC, S], fr, tag=f"QQ{p}", name=f"QQ{p}") for p in range(2)]
            KK = [qkvp.tile([KC, S], fr, tag=f"KK{p}", name=f"KK{p}") for p in range(2)]
            # V[2p+hh]: [128, 16*65]; chunk kc at cols kc*65..+64; col 64: 1.0
            V = [qkvp.tile([KC, NKC * (DH + 1)], fr, tag=f"V{v}", name=f"V{v}")
                 for v in range(4)]
            vones_sb = mp.tile([KC, NKC], fr, tag="vones_sb")
            nc.sync.dma_start(out=vones_sb[:], in_=vones[:])
            for v in range(4):
                vv = V[v][:].rearrange("p (k c) -> p k c", c=DH + 1)
                nc.vector.tensor_copy(vv[:, :, DH], vones_sb[:])
            # O[p]: rows 0:64 = head 2c out^T (normalized), 64:128 = head 2c+1
            O = [op.tile([KC, S], fr, tag=f"O{p}", name=f"O{p}") for p in range(2)]

            # ---- phase 1: projections ------------------------------------
            with (
                tc.tile_pool(name="pjw", bufs=1) as wp,
                tc.tile_pool(name="xbuf", bufs=24) as xp,
                tc.tile_pool(name="psum_proj", bufs=1, space="PSUM") as pp,
            ):
                wqk_t = [wp.tile([KC, 2 * KC], fr, tag=f"wqk{k}", name=f"wqk{k}")
                         for k in range(NDC)]
                wv_t = [wp.tile([KC, 2 * KC], fr, tag=f"wv{k}", name=f"wv{k}")
                        for k in range(NDC)]
                for k in range(NDC):
                    nc.sync.dma_start(out=wqk_t[k][:], in_=wqk[k])
                    nc.sync.dma_start(out=wv_t[k][:], in_=wv[k])

                for sblk in range(NSB):
                    for bb in range(B):
                        xs = []
                        for k in range(NDC):
                            xtl = xp.tile([KC, SB], fr, tag="xt")
                            nc.sync.dma_start(out=xtl[:], in_=xt[bb, sblk, k])
                            xs.append(xtl)
                        # m-chunk 0 -> QQ[bb], 1 -> KK[bb]
                        for m in range(2):
                            ps = pp.tile([KC, SB], f32, tag="ps_qk", bufs=4)
                            for k in range(NDC):
                                nc.tensor.matmul(
                                    ps[:],
                                    wqk_t[k][:, m * KC:(m + 1) * KC],
                                    xs[k][:],
                                    start=(k == 0),
                                    stop=(k == NDC - 1),
                                )
                            dest = (QQ if m == 0 else KK)[bb]
                            nc.vector.tensor_scalar_add(
                                dest[:, sblk * SB:(sblk + 1) * SB], ps[:],
                                bqk_t[m][:],
                            )
                        # V natural: lhsT = x^T chunk; rhs = Wv (zero-padded
                        # to N=256 so f32r streams at full rate)
                        for sc in range(SB // KC):
                            ps = pp.tile([KC, 2 * KC], f32, tag="ps_v", bufs=4)
                            for k in range(NDC):
                                nc.tensor.matmul(
                                    ps[:],
                                    xs[k][:, sc * KC:(sc + 1) * KC],
                                    wv_t[k][:],
                                    start=(k == 0),
                                    stop=(k == NDC - 1),
                                )
                            kcg = sblk * (SB // KC) + sc
                            for hh in range(2):
                                nc.vector.tensor_add(
                                    V[2 * bb + hh][:, kcg * (DH + 1):
                                                   kcg * (DH + 1) + DH],
                                    ps[:, hh * DH:(hh + 1) * DH],
                                    bv_bc[:, hh * DH:(hh + 1) * DH],
                                )

            # ---- phase 2: attention --------------------------------------
            with (
                tc.tile_pool(name="pbuf", bufs=1) as pb,
                tc.tile_pool(name="psum_att", bufs=1, space="PSUM") as pa,
            ):
                for qblk in range(NSB):
                    nkc = 4 * (qblk + 1)  # causal: k-chunks 0..nkc-1
                    for p in range(B):
                        # P[kc]: [128, 1024]; cols hh*512.. hold head hh
                        P = [
                            pb.tile([KC, 2 * SB], fr, tag=f"P{kc}",
                                    name=f"P{kc}_{p}_{qblk}",
                                    bufs=(2 if kc < 11 else 1))
                            for kc in range(nkc)
                        ]
                        for kc in range(nkc):
                            d = kc - 4 * qblk
                            # causal: columns < 128*d are fully masked; skip
                            # them in the matmul/exp where the speed holds up
                            c0 = min(KC * max(d, 0), 2 * KC)
                            ps = pa.tile([KC, 2 * SB], f32, tag="ps_s", bufs=3)
                            for hh in range(2):  # row-tiled head pair
                                r0 = hh * DH
                                nc.tensor.matmul(
                                    ps[:, hh * SB + c0:(hh + 1) * SB],
                                    KK[p][r0:r0 + DH, kc * KC:(kc + 1) * KC],
                                    QQ[p][r0:r0 + DH,
                                         qblk * SB + c0:(qblk + 1) * SB],
                                    start=True,
                                    stop=True,
                                )
                            ps3 = ps[:].rearrange("p (h f) -> p h f", h=2)
                            pd3 = P[kc][:].rearrange("p (h f) -> p h f", h=2)
                            e0 = KC * max(d, 0)
                            nc.scalar.activation(
                                pd3[:, :, e0:SB],
                                ps3[:, :, e0:SB],
                                mybir.ActivationFunctionType.Exp,
                                scale=1.0 / float(np.sqrt(DH)),
                            )
                            if d >= 0:  # diagonal chunk: zero where k > q
                                nc.gpsimd.affine_select(
                                    out=pd3[:, :, :],
                                    in_=pd3[:, :, :],
                                    pattern=[[0, 2], [1, SB]],
                                    compare_op=mybir.AluOpType.is_ge,
                                    fill=0.0,
                                    base=-KC * d,
                                    channel_multiplier=-1,
                                )
                        pos = [pa.tile([DH + 1, SB], f32, tag=f"ps_av{hh}",
                                       bufs=1, name=f"po{hh}_{p}_{qblk}")
                               for hh in range(2)]
                        for kc in range(nkc):
                            d = kc - 4 * qblk
                            c0 = min(KC * max(d, 0), 2 * KC)
                            for hh in range(2):
                                nc.tensor.matmul(
                                    pos[hh][:, c0:SB],
                                    V[2 * p + hh][:, kc * (DH + 1):
                                                  (kc + 1) * (DH + 1)],
                                    P[kc][:, hh * SB + c0:(hh + 1) * SB],
                                    start=(kc == 0),
                                    stop=(kc == nkc - 1),
                                )
                        for hh in range(2):
                            po = pos[hh]
                            # free the psum bank immediately; normalize later
                            avst = ep.tile([DH + 1, SB], f32, tag="avst", bufs=4)
                            nc.vector.tensor_copy(avst[:], po[:])
                            rden = ep.tile([1, SB], f32, tag="rden", bufs=2)
                            nc.vector.reciprocal(rden[:], avst[DH:DH + 1, :])
                            rden_bc = ep.tile([DH, SB], f32, tag="rden_bc", bufs=2)
                            nc.gpsimd.partition_broadcast(
                                out_ap=rden_bc[:], in_ap=rden[:]
                            )
                            r0 = hh * DH
                            nc.vector.tensor_mul(
                                O[p][r0:r0 + DH, qblk * SB:(qblk + 1) * SB],
                                avst[0:DH, :],
                                rden_bc[:],
                            )
                        # stage this (batch, quarter) block for the AllToAll
                        nc.sync.dma_start(
                            out=a2a_in[4 * p + qblk],
                            in_=O[p][:, qblk * SB:(qblk + 1) * SB],
                        )

            # ---- phase 3: head exchange + output projection --------------
            nc.gpsimd.collective_compute(
                "AllToAll",
                mybir.AluOpType.bypass,
                replica_groups=[[0, 1, 2, 3, 4, 5, 6, 7]],
                ins=[a2a_in[:]],
                outs=[a2a_out[:]],
            )
            with (
                tc.tile_pool(name="wout_pool", bufs=1) as wop,
                tc.tile_pool(name="recv", bufs=1) as rp,
                tc.tile_pool(name="psum_out", bufs=1, space="PSUM") as pu,
            ):
                wout_t = [wop.tile([KC, D], fr, tag=f"wo{k}", name=f"wo{k}")
                          for k in range(NDC)]
                for k in range(NDC):
                    nc.sync.dma_start(out=wout_t[k][:], in_=wout[k])
                # a2a_out block i = heads (2i, 2i+1) of my batch for my
                # quarter -> flat [1024, 512] = attnout^T in global head order
                recv = [rp.tile([KC, SB], fr, tag=f"rc{k}", name=f"rc{k}")
                        for k in range(NDC)]
                for k in range(NDC):
                    nc.sync.dma_start(out=recv[k][:], in_=a2a_out[k])
                for sc in range(SB // KC):
                    for nb in range(D // SB):
                        ps = pu.tile([KC, SB], f32, tag="ps_o", bufs=4)
                        for k in range(NDC):
                            nc.tensor.matmul(
                                ps[:],
                                recv[k][:, sc * KC:(sc + 1) * KC],
                                wout_t[k][:, nb * SB:(nb + 1) * SB],
                                start=(k == 0),
                                stop=(k == NDC - 1),
                            )
                        ot = ep.tile([KC, SB], f32, tag="osb", bufs=4)
                        nc.vector.tensor_add(
                            ot[:], ps[:], bo_bc[:, nb * SB:(nb + 1) * SB]
                        )
                        nc.sync.dma_start(
                            out=out_ext[sc * KC:(sc + 1) * KC,
                                        nb * SB:(nb + 1) * SB],
                            in_=ot[:],
                        )

    nc.compile()
    return nc


def _get_program():
    global _compiled
    if _compiled is None:
        _compiled = _build()
    return _compiled


def _shard_inputs(x, Wqkv, bqkv, Wout, bout):
    """Build the 8 per-core input maps (all host-side numpy)."""
    x = np.ascontiguousarray(x, dtype=np.float32)
    Wqkv = np.asarray(Wqkv, dtype=np.float32)
    bqkv = np.asarray(bqkv, dtype=np.float32)
    Wout = np.asarray(Wout, dtype=np.float32)
    bout = np.ascontiguousarray(np.asarray(bout, dtype=np.float32))

    Wq = Wqkv[:, 0 * D:1 * D]
    Wk = Wqkv[:, 1 * D:2 * D]
    Wv_full = Wqkv[:, 2 * D:3 * D]
    bq = bqkv[0 * D:1 * D]
    bk = bqkv[1 * D:2 * D]
    bv_full = bqkv[2 * D:3 * D]

    # shared across all cores
    xt = np.ascontiguousarray(
        x.transpose(0, 2, 1)                      # [B, D, S]
         .reshape(B, D, NSB, SB).transpose(0, 2, 1, 3)
         .reshape(B, NSB, NDC, KC, SB)
    )
    wout_b = np.ascontiguousarray(Wout.reshape(NDC, KC, D))
    vones = np.ones((KC, NKC), dtype=np.float32)

    in_maps = []
    for c in range(NCORES):
        ha, hb = 2 * c, 2 * c + 1
        wqk_c = np.ascontiguousarray(np.concatenate(
            [Wq[:, ha * DH:(ha + 1) * DH], Wq[:, hb * DH:(hb + 1) * DH],
             Wk[:, ha * DH:(ha + 1) * DH], Wk[:, hb * DH:(hb + 1) * DH]],
            axis=1).reshape(NDC, KC, 2 * KC))
        bqk_c = np.ascontiguousarray(np.concatenate(
            [bq[ha * DH:(ha + 1) * DH], bq[hb * DH:(hb + 1) * DH],
             bk[ha * DH:(ha + 1) * DH], bk[hb * DH:(hb + 1) * DH]]))
        # Wv zero-padded to 256 columns so the V matmul moving dim is 256
        wv_c = np.zeros((D, 2 * KC), dtype=np.float32)
        wv_c[:, 0:DH] = Wv_full[:, ha * DH:(ha + 1) * DH]
        wv_c[:, DH:2 * DH] = Wv_full[:, hb * DH:(hb + 1) * DH]
        wv_c = np.ascontiguousarray(wv_c.reshape(NDC, KC, 2 * KC))
        bv_c = np.ascontiguousarray(np.concatenate(
            [bv_full[ha * DH:(ha + 1) * DH], bv_full[hb * DH:(hb + 1) * DH]]))
        in_maps.append({
            "xt": xt, "wqk": wqk_c, "wv": wv_c, "wout": wout_b,
            "bqk": bqk_c, "bv": bv_c, "bo": bout, "vones": vones,
        })
    return in_maps


def run(inputs, trace=False, trace_kwargs=None):
    nc = _get_program()
    in_maps = _shard_inputs(**inputs)
    res = run_bass_kernel_spmd(
        nc, in_maps, list(range(NCORES)), trace=trace,
        **(trace_kwargs or {}),
    )
    out = np.empty((B, S, D), dtype=np.float32)
    for c in range(NCORES):
        b = c // 4
        r0 = SB * (c % 4)
        out[b, r0:r0 + SB, :] = res.results[c]["out"]
    return out, res


def kernel(**inputs):
    out, _ = run(inputs)
    return out


# revision 14
# speedup vs baseline: 1.6699x; 1.0017x over previous
"""Causal multi-head attention (B=2, S=2048, D=1024, H=16) on 8 trn2 cores.

Sharding: core c handles heads {2c, 2c+1} of BOTH batches (4 (b,h) pairs).
Per core:
  - project host-pretransposed x_b^T [D, S] (both batches) through the
    core's Wqkv column slice into Q^T/K^T head-pair tiles and V (natural
    layout, with a fused ones-column that makes the AV matmul emit softmax
    denominators),
  - causal attention per (batch, head) in transposed layout: scores^T =
    K Q^T chunks (PE row-tiled head pairs), exp on ScalarE, causal diagonal
    masks via gpsimd affine_select, A^T V on PE,
  - one 8-wide AllToAll redistributes head outputs so core c holds ALL 16
    heads of batch c//4 for sequence quarter c%4,
  - local projection through the full Wout emits final rows
    512*(c%4) .. +512 of batch c//4.
Host assembles the 8 [512, 1024] shards into (2, 2048, 1024).

Matmuls run in float32r (TF32-like single-pass PE mode, ~1e-3 rel err,
4x faster than true fp32). The PE rounds f32r inputs internally, so DRAM
inputs are declared float32r and DMA'd with the fast HW-DGE path with no
pre-rounding. Set _USE_F32R = False for full fp32.
"""

import sys

for _p in ("/opt/trn_rl_repo", "/opt/pypackages"):
    if _p not in sys.path:
        sys.path.insert(0, _p)

import numpy as np

import concourse.bass as bass
import concourse.mybir as mybir
import concourse.tile as tile
from concourse import bacc
from concourse.bass_utils import run_bass_kernel_spmd

B = 2
S = 2048
D = 1024
H = 16
DH = 64
NCORES = 8
SB = 512           # q block (matmul moving dim)
KC = 128           # k chunk (contraction tile)
NSB = S // SB      # 4 q-blocks
NKC = S // KC      # 16 k-chunks
NDC = D // KC      # 8 contraction chunks for the projections

_USE_F32R = True

_compiled = None


def _build():
    f32 = mybir.dt.float32
    bf16 = mybir.dt.bfloat16
    fr = mybir.dt.float32r if _USE_F32R else f32
    nc = bacc.Bacc(None, target_bir_lowering=False)

    # host-blocked inputs: every [128, N] tile is contiguous in DRAM.
    # Matmul inputs are declared float32r: same 4-byte data, PE rounds
    # internally, and plain (non-casting) sync DMA is allowed.
    xt = nc.declare_dram_parameter("xt", [B, NSB, NDC, KC, SB], fr, isOutput=False)
    wqk = nc.declare_dram_parameter("wqk", [NDC, KC, 2 * KC], fr, isOutput=False)
    wv = nc.declare_dram_parameter("wv", [NDC, KC, 2 * KC], fr, isOutput=False)
    wout = nc.declare_dram_parameter("wout", [NDC, KC, D], fr, isOutput=False)
    bqk = nc.declare_dram_parameter("bqk", [2 * KC], f32, isOutput=False)
    bv = nc.declare_dram_parameter("bv", [2 * DH], f32, isOutput=False)
    bo = nc.declare_dram_parameter("bo", [D], f32, isOutput=False)
    vones = nc.declare_dram_parameter("vones", [KC, NKC], bf16, isOutput=False)
    out_ext = nc.declare_dram_parameter("out", [SB, D], f32, isOutput=True)

    # AllToAll staging: block t -> core t gets my heads of batch t//4 for
    # s-quarter t%4.
    a2a_in = nc.dram_tensor("a2a_in", [NCORES, KC, SB], fr)
    a2a_out = nc.dram_tensor("a2a_out", [NCORES, KC, SB], fr)

    with tile.TileContext(nc) as tc:
        with (
            tc.tile_pool(name="qkv", bufs=1) as qkvp,
            tc.tile_pool(name="obuf", bufs=1) as op,
            tc.tile_pool(name="misc", bufs=1) as mp,
            tc.tile_pool(name="evict", bufs=1) as ep,
        ):
            # ---- small constants -----------------------------------------
            bqk_t = [mp.tile([KC, 1], f32, tag=f"bqk{m}", name=f"bqk{m}")
                     for m in range(2)]
            for m in range(2):
                nc.sync.dma_start(
                    out=bqk_t[m][:],
                    in_=bqk[m * KC:(m + 1) * KC].rearrange("(p o) -> p o", o=1),
                )
            bv_row = mp.tile([1, 2 * DH], f32, tag="bv_row")
            nc.sync.dma_start(out=bv_row[:], in_=bv.rearrange("(o f) -> o f", o=1))
            bv_bc = mp.tile([KC, 2 * DH], f32, tag="bv_bc")
            nc.gpsimd.partition_broadcast(out_ap=bv_bc[:], in_ap=bv_row[:])
            bo_row = mp.tile([1, D], f32, tag="bo_row")
            nc.sync.dma_start(out=bo_row[:], in_=bo.rearrange("(o f) -> o f", o=1))
            bo_bc = mp.tile([KC, D], f32, tag="bo_bc")
            nc.gpsimd.partition_broadcast(out_ap=bo_bc[:], in_ap=bo_row[:])

            # ---- persistent activations ----------------------------------
            # pair p = batch p with heads (2c, 2c+1).
            # QQ[p]: rows 0:64 = Q^T of head 2c, rows 64:128 = head 2c+1
            QQ = [qkvp.tile([KC, S], fr, tag=f"QQ{p}", name=f"QQ{p}") for p in range(2)]
            KK = [qkvp.tile([KC, S], fr, tag=f"KK{p}", name=f"KK{p}") for p in range(2)]
            # V[2p+hh]: [128, 16*65]; chunk kc at cols kc*65..+64; col 64: 1.0
            V = [qkvp.tile([KC, NKC * (DH + 1)], bf16, tag=f"V{v}", name=f"V{v}")
                 for v in range(4)]
            vones_sb = mp.tile([KC, NKC], bf16, tag="vones_sb")
            nc.sync.dma_start(out=vones_sb[:], in_=vones[:])
            for v in range(4):
                vv = V[v][:].rearrange("p (k c) -> p k c", c=DH + 1)
                nc.vector.tensor_copy(vv[:, :, DH], vones_sb[:])
            # O[p]: rows 0:64 = head 2c out^T (normalized), 64:128 = head 2c+1
            O = [op.tile([KC, S], fr, tag=f"O{p}", name=f"O{p}") for p in range(2)]

            # ---- phase 1: projections ------------------------------------
            with (
                tc.tile_pool(name="pjw", bufs=1) as wp,
                tc.tile_pool(name="xbuf", bufs=24) as xp,
                tc.tile_pool(name="psum_proj", bufs=1, space="PSUM") as pp,
            ):
                wqk_t = [wp.tile([KC, 2 * KC], fr, tag=f"wqk{k}", name=f"wqk{k}")
                         for k in range(NDC)]
                wv_t = [wp.tile([KC, 2 * KC], fr, tag=f"wv{k}", name=f"wv{k}")
                        for k in range(NDC)]
                for k in range(NDC):
                    nc.sync.dma_start(out=wqk_t[k][:], in_=wqk[k])
                    nc.sync.dma_start(out=wv_t[k][:], in_=wv[k])

                for sblk in range(NSB):
                    for bb in range(B):
                        xs = []
                        for k in range(NDC):
                            xtl = xp.tile([KC, SB], fr, tag="xt")
                            nc.sync.dma_start(out=xtl[:], in_=xt[bb, sblk, k])
                            xs.append(xtl)
                        # m-chunk 0 -> QQ[bb], 1 -> KK[bb]
                        for m in range(2):
                            ps = pp.tile([KC, SB], f32, tag="ps_qk", bufs=4)
                            for k in range(NDC):
                                nc.tensor.matmul(
                                    ps[:],
                                    wqk_t[k][:, m * KC:(m + 1) * KC],
                                    xs[k][:],
                                    start=(k == 0),
                                    stop=(k == NDC - 1),
                                )
                            dest = (QQ if m == 0 else KK)[bb]
                            nc.vector.tensor_scalar_add(
                                dest[:, sblk * SB:(sblk + 1) * SB], ps[:],
                                bqk_t[m][:],
                            )
                        # V natural: lhsT = x^T chunk; rhs = Wv (zero-padded
                        # to N=256 so f32r streams at full rate)
                        for sc in range(SB // KC):
                            ps = pp.tile([KC, 2 * KC], f32, tag="ps_v", bufs=4)
                            for k in range(NDC):
                                nc.tensor.matmul(
                                    ps[:],
                                    xs[k][:, sc * KC:(sc + 1) * KC],
                                    wv_t[k][:],
                                    start=(k == 0),
                                    stop=(k == NDC - 1),
                                )
                            kcg = sblk * (SB // KC) + sc
                            for hh in range(2):
                                nc.vector.tensor_add(
                                    V[2 * bb + hh][:, kcg * (DH + 1):
                                                   kcg * (DH + 1) + DH],
                                    ps[:, hh * DH:(hh + 1) * DH],
                                    bv_bc[:, hh * DH:(hh + 1) * DH],
                                )

            # ---- phase 2: attention --------------------------------------
            with (
                tc.tile_pool(name="pbuf", bufs=1) as pb,
                tc.tile_pool(name="psum_att", bufs=1, space="PSUM") as pa,
            ):
                for qblk in range(NSB):
                    nkc = 4 * (qblk + 1)  # causal: k-chunks 0..nkc-1
                    for p in range(B):
                        # P[kc]: [128, 1024]; cols hh*512.. hold head hh
                        P = [
                            pb.tile([KC, 2 * SB], bf16, tag=f"P{kc}",
                                    name=f"P{kc}_{p}_{qblk}", bufs=2)
                            for kc in range(nkc)
                        ]
                        for kc in range(nkc):
                            d = kc - 4 * qblk
                            # causal: columns < 128*d are fully masked; skip
                            # them in the matmul/exp where the speed holds up
                            c0 = min(KC * max(d, 0), 2 * KC)
                            ps = pa.tile([KC, 2 * SB], f32, tag="ps_s", bufs=3)
                            for hh in range(2):  # row-tiled head pair
                                r0 = hh * DH
                                nc.tensor.matmul(
                                    ps[:, hh * SB + c0:(hh + 1) * SB],
                                    KK[p][r0:r0 + DH, kc * KC:(kc + 1) * KC],
                                    QQ[p][r0:r0 + DH,
                                         qblk * SB + c0:(qblk + 1) * SB],
                                    start=True,
                                    stop=True,
                                )
                            ps3 = ps[:].rearrange("p (h f) -> p h f", h=2)
                            pd3 = P[kc][:].rearrange("p (h f) -> p h f", h=2)
                            e0 = KC * max(d, 0)
                            nc.scalar.activation(
                                pd3[:, :, e0:SB],
                                ps3[:, :, e0:SB],
                                mybir.ActivationFunctionType.Exp,
                                scale=1.0 / float(np.sqrt(DH)),
                            )
                            if d >= 0:  # diagonal chunk: zero where k > q
                                nc.gpsimd.affine_select(
                                    out=pd3[:, :, :],
                                    in_=pd3[:, :, :],
                                    pattern=[[0, 2], [1, SB]],
                                    compare_op=mybir.AluOpType.is_ge,
                                    fill=0.0,
                                    base=-KC * d,
                                    channel_multiplier=-1,
                                )
                        pos = [pa.tile([DH + 1, SB], f32, tag=f"ps_av{hh}",
                                       bufs=1, name=f"po{hh}_{p}_{qblk}")
                               for hh in range(2)]
                        for kc in range(nkc):
                            d = kc - 4 * qblk
                            c0 = min(KC * max(d, 0), 2 * KC)
                            for hh in range(2):
                                nc.tensor.matmul(
                                    pos[hh][:, c0:SB],
                                    V[2 * p + hh][:, kc * (DH + 1):
                                                  (kc + 1) * (DH + 1)],
                                    P[kc][:, hh * SB + c0:(hh + 1) * SB],
                                    start=(kc == 0),
                                    stop=(kc == nkc - 1),
                                )
                        for hh in range(2):
                            po = pos[hh]
                            # free the psum bank immediately; normalize later
                            avst = ep.tile([DH + 1, SB], f32, tag="avst", bufs=4)
                            nc.vector.tensor_copy(avst[:], po[:])
                            rden = ep.tile([1, SB], f32, tag="rden", bufs=2)
                            nc.vector.reciprocal(rden[:], avst[DH:DH + 1, :])
                            rden_bc = ep.tile([DH, SB], f32, tag="rden_bc", bufs=2)
                            nc.gpsimd.partition_broadcast(
                                out_ap=rden_bc[:], in_ap=rden[:]
                            )
                            r0 = hh * DH
                            nc.vector.tensor_mul(
                                O[p][r0:r0 + DH, qblk * SB:(qblk + 1) * SB],
                                avst[0:DH, :],
                                rden_bc[:],
                            )
                        # stage this (batch, quarter) block for the AllToAll
                        nc.sync.dma_start(
                            out=a2a_in[4 * p + qblk],
                            in_=O[p][:, qblk * SB:(qblk + 1) * SB],
                        )

            # ---- phase 3: head exchange + output projection --------------
            nc.gpsimd.collective_compute(
                "AllToAll",
                mybir.AluOpType.bypass,
                replica_groups=[[0, 1, 2, 3, 4, 5, 6, 7]],
                ins=[a2a_in[:]],
                outs=[a2a_out[:]],
            )
            with (
                tc.tile_pool(name="wout_pool", bufs=1) as wop,
                tc.tile_pool(name="recv", bufs=1) as rp,
                tc.tile_pool(name="psum_out", bufs=1, space="PSUM") as pu,
            ):
                wout_t = [wop.tile([KC, D], fr, tag=f"wo{k}", name=f"wo{k}")
                          for k in range(NDC)]
                for k in range(NDC):
                    nc.sync.dma_start(out=wout_t[k][:], in_=wout[k])
                # a2a_out block i = heads (2i, 2i+1) of my batch for my
                # quarter -> flat [1024, 512] = attnout^T in global head order
                recv = [rp.tile([KC, SB], fr, tag=f"rc{k}", name=f"rc{k}")
                        for k in range(NDC)]
                for k in range(NDC):
                    nc.sync.dma_start(out=recv[k][:], in_=a2a_out[k])
                for sc in range(SB // KC):
                    for nb in range(D // SB):
                        ps = pu.tile([KC, SB], f32, tag="ps_o", bufs=4)
                        for k in range(NDC):
                            nc.tensor.matmul(
                                ps[:],
                                recv[k][:, sc * KC:(sc + 1) * KC],
                                wout_t[k][:, nb * SB:(nb + 1) * SB],
                                start=(k == 0),
                                stop=(k == NDC - 1),
                            )
                        ot = ep.tile([KC, SB], f32, tag="osb", bufs=4)
                        nc.vector.tensor_add(
                            ot[:], ps[:], bo_bc[:, nb * SB:(nb + 1) * SB]
                        )
                        nc.sync.dma_start(
                            out=out_ext[sc * KC:(sc + 1) * KC,
                                        nb * SB:(nb + 1) * SB],
                            in_=ot[:],
                        )

    nc.compile()
    return nc


def _get_program():
    global _compiled
    if _compiled is None:
        _compiled = _build()
    return _compiled


def _shard_inputs(x, Wqkv, bqkv, Wout, bout):
    """Build the 8 per-core input maps (all host-side numpy)."""
    x = np.ascontiguousarray(x, dtype=np.float32)
    Wqkv = np.asarray(Wqkv, dtype=np.float32)
    bqkv = np.asarray(bqkv, dtype=np.float32)
    Wout = np.asarray(Wout, dtype=np.float32)
    bout = np.ascontiguousarray(np.asarray(bout, dtype=np.float32))

    Wq = Wqkv[:, 0 * D:1 * D]
    Wk = Wqkv[:, 1 * D:2 * D]
    Wv_full = Wqkv[:, 2 * D:3 * D]
    bq = bqkv[0 * D:1 * D]
    bk = bqkv[1 * D:2 * D]
    bv_full = bqkv[2 * D:3 * D]

    # shared across all cores
    xt = np.ascontiguousarray(
        x.transpose(0, 2, 1)                      # [B, D, S]
         .reshape(B, D, NSB, SB).transpose(0, 2, 1, 3)
         .reshape(B, NSB, NDC, KC, SB)
    )
    wout_b = np.ascontiguousarray(Wout.reshape(NDC, KC, D))
    import ml_dtypes
    vones = np.ones((KC, NKC), dtype=ml_dtypes.bfloat16)

    in_maps = []
    for c in range(NCORES):
        ha, hb = 2 * c, 2 * c + 1
        wqk_c = np.ascontiguousarray(np.concatenate(
            [Wq[:, ha * DH:(ha + 1) * DH], Wq[:, hb * DH:(hb + 1) * DH],
             Wk[:, ha * DH:(ha + 1) * DH], Wk[:, hb * DH:(hb + 1) * DH]],
            axis=1).reshape(NDC, KC, 2 * KC))
        bqk_c = np.ascontiguousarray(np.concatenate(
            [bq[ha * DH:(ha + 1) * DH], bq[hb * DH:(hb + 1) * DH],
             bk[ha * DH:(ha + 1) * DH], bk[hb * DH:(hb + 1) * DH]]))
        # Wv zero-padded to 256 columns so the V matmul moving dim is 256
        wv_c = np.zeros((D, 2 * KC), dtype=np.float32)
        wv_c[:, 0:DH] = Wv_full[:, ha * DH:(ha + 1) * DH]
        wv_c[:, DH:2 * DH] = Wv_full[:, hb * DH:(hb + 1) * DH]
        wv_c = np.ascontiguousarray(wv_c.reshape(NDC, KC, 2 * KC))
        bv_c = np.ascontiguousarray(np.concatenate(
            [bv_full[ha * DH:(ha + 1) * DH], bv_full[hb * DH:(hb + 1) * DH]]))
        in_maps.append({
            "xt": xt, "wqk": wqk_c, "wv": wv_c, "wout": wout_b,
            "bqk": bqk_c, "bv": bv_c, "bo": bout, "vones": vones,
        })
    return in_maps


def run(inputs, trace=False, trace_kwargs=None):
    nc = _get_program()
    in_maps = _shard_inputs(**inputs)
    res = run_bass_kernel_spmd(
        nc, in_maps, list(range(NCORES)), trace=trace,
        **(trace_kwargs or {}),
    )
    out = np.empty((B, S, D), dtype=np.float32)
    for c in range(NCORES):
        b = c // 4
        r0 = SB * (c % 4)
        out[b, r0:r0 + SB, :] = res.results[c]["out"]
    return out, res


def kernel(**inputs):
    out, _ = run(inputs)
    return out
